# revision 1
# baseline (speedup 1.0000x reference)
"""Multi-head attention Trainium2 Bass kernel, sharded over 8 NeuronCores.

Problem: B=4, S=2048, D=1024, H=16 heads (DK=64), fp32, random 0/1 mask.

Sharding (data-parallel batch x tensor-parallel heads):
  core c handles batch b = c // 2, head-group hg = c % 2 (8 heads = 512 dims).
  Each core computes Q/K/V projections for its head-group, masked softmax
  attention for its 8 heads, and a partial output projection over its 512
  contraction dims. The host sums the two partials per batch (the "all-reduce"
  is a host-side pairwise add since we gather outputs anyway).

On-device layout (per core):
  All matmuls run as float32r (full PE rate at N=512, ~1e-3 relative precision)
  except attention-weights @ V which runs in bf16 (E and V tiles), because the
  mask multiply on the vector engine needs bf16 for its 2x mode.

  K^T is kept resident [512, 2048] (head dim on partitions); Q^T is computed
  per 512-wide q-chunk inside the attention loop (hides the Q projection under
  the exp-bound phase). Scores come out transposed, S^T = [k, q], with the two
  heads of a pair row-packed into disjoint halves of the PE array. The softmax
  sum over k rides the P@V matmul as a ones column appended to V (row 64 of
  the C' accumulator = sum_k E_masked). exp() runs on the scalar engine out of
  PSUM (2 banks per ACTIVATE); the mask multiply runs on the vector engine in
  bf16 2x mode; 1/Z = scalar-engine partition-shifting copy (PSUM row 64 ->
  SBUF row 0) + reciprocal_approx_fast + gpsimd partition_broadcast. The
  output projection of chunk qc is emitted inside chunk qc+1's loop so its
  matmuls never stall the in-order PE queue.

  Output is produced transposed ([1024, 2048] per core); host re-transposes
  and sums the two head-group partials per batch.
"""
import numpy as np

import concourse.bass as bass
import concourse.mybir as mybir
import concourse.tile as tile
from concourse import bacc

B, S, D, H = 4, 2048, 1024, 16
DK = D // H          # 64
NCORES = 8
HG = 2               # head groups (tensor-parallel factor per batch)
HPG = H // HG        # 8 heads per core
DH = D // HG         # 512 head dims per core
QCN = 4              # q chunks
QCS = S // QCN       # 512
KT = S // 128        # 16 k tiles
DT = D // 128        # 8 contraction tiles for projections
F32 = mybir.dt.float32
F32R = mybir.dt.float32r
BF16 = mybir.dt.bfloat16

# k-tile grouping for the exp pass (PSUM banks per S^T buffer)
EXP_GROUPS = [3, 3, 3, 3, 2, 2]
assert sum(EXP_GROUPS) == KT


def r(ap):
    """Matmul operands are stored as float32r already; identity."""
    return ap


def build_nc():
    nc = bacc.Bacc(None)
    xqT = nc.declare_dram_parameter("xqT", [D, S], F32R, isOutput=False)
    xkT = nc.declare_dram_parameter("xkT", [D, S], F32R, isOutput=False)
    xvT = nc.declare_dram_parameter("xvT", [D, S], F32R, isOutput=False)
    maskT = nc.declare_dram_parameter("maskT", [S, S], BF16, isOutput=False)
    wqT = nc.declare_dram_parameter("wqT", [D, DH], F32R, isOutput=False)
    wkT = nc.declare_dram_parameter("wkT", [D, DH], F32R, isOutput=False)
    wvT = nc.declare_dram_parameter("wvT", [D, DH], F32R, isOutput=False)
    woT = nc.declare_dram_parameter("woT", [DH, D], F32R, isOutput=False)
    bq2 = nc.declare_dram_parameter("bq2", [128, DH // 128], F32, isOutput=False)
    bk2 = nc.declare_dram_parameter("bk2", [128, DH // 128], F32, isOutput=False)
    vr2 = nc.declare_dram_parameter("vr2", [128, D // 128], F32, isOutput=False)
    outT = nc.declare_dram_parameter("outT", [D, S], F32, isOutput=True)

    with tile.TileContext(nc) as tc:
        with tc.tile_pool(name="persist", bufs=1) as pp:
            # ---- persistent tiles ----
            wo_full = pp.tile([128, DH // 128, D], F32R, tag="wo", name="wo_full")
            wo_sb = [wo_full[:, i, :] for i in range(DH // 128)]
            bias_sb = pp.tile([128, 2 * (DH // 128) + D // 128], F32, tag="bias",
                              name="bias_sb")
            bq_sb = bias_sb[:, 0:DH // 128]
            bk_sb = bias_sb[:, DH // 128:2 * (DH // 128)]
            vr_sb = bias_sb[:, 2 * (DH // 128):]
            wq_full = pp.tile([128, DT, DH], F32R, tag="wq", name="wq_full")
            wq_sb = [wq_full[:, i, :] for i in range(DT)]
            kt_sb = [pp.tile([128, S], F32R, tag=f"kt{i}", name=f"kt{i}")
                     for i in range(DH // 128)]
            v_full = pp.tile([128, KT, HPG * 65], BF16, tag="v", name="v_full")
            v_sb = [v_full[:, i, :] for i in range(KT)]

            # ---- phase A: projections ----
            with (
                tc.tile_pool(name="w_a", bufs=1) as wpool,
                tc.tile_pool(name="x_a", bufs=2) as xpool,
                tc.tile_pool(name="ps_a", bufs=6, space="PSUM") as pspool,
            ):
                wk_full = wpool.tile([128, DT, DH], F32R, tag="wk", name="wk_full")
                wv_full = wpool.tile([128, DT, DH], F32R, tag="wv", name="wv_full")
                wk_sb = [wk_full[:, i, :] for i in range(DT)]
                wv_sb = [wv_full[:, i, :] for i in range(DT)]
                # wk first on the sync queue (K projection runs first)
                for i in range(DT):
                    nc.sync.dma_start(wk_sb[i][:], wkT[i * 128:(i + 1) * 128, :])
                nc.gpsimd.dma_start(bq_sb[:, :], bq2[:])
                nc.gpsimd.dma_start(bk_sb[:, :], bk2[:])
                nc.gpsimd.dma_start(vr_sb[:, :], vr2[:])
                for i in range(KT):
                    ones_cols = v_sb[i].rearrange("p (h c) -> p h c", h=HPG)[:, :, 64:65]
                    nc.gpsimd.memset(ones_cols, 1.0)

                def qk_proj(xT_dram, w_sb, dst_tiles, scale, bias_sb, wtag,
                            first_on_pool=False):
                    for qc in range(QCN):
                        x_t = xpool.tile([128, DT, QCS], F32R, tag="x", name=f"x_{wtag}",
                                         bufs=2)
                        xs = xT_dram[:, qc * QCS:(qc + 1) * QCS].rearrange(
                            "(t p) s -> p t s", p=128)
                        for i in range(DT):
                            eng = nc.gpsimd if (first_on_pool or i % 2) else nc.sync
                            eng.dma_start(x_t[:, i:i + 1, :], xs[:, i:i + 1, :])
                        first_on_pool = False
                        ps_l = [pspool.tile([128, QCS], F32, tag="ps",
                                            name=f"ps_{wtag}{dt}")
                                for dt in range(DH // 128)]
                        for i in range(DT):
                            for dt in range(DH // 128):
                                nc.tensor.matmul(
                                    ps_l[dt][:], w_sb[i][:, dt * 128:(dt + 1) * 128],
                                    x_t[:, i, :], start=(i == 0), stop=(i == DT - 1))
                        for dt in range(DH // 128):
                            nc.vector.tensor_scalar(
                                dst_tiles[dt][:, qc * QCS:(qc + 1) * QCS], ps_l[dt][:],
                                scale, bias_sb[:, dt:dt + 1],
                                mybir.AluOpType.mult, mybir.AluOpType.add)

                qk_proj(xkT, wk_sb, kt_sb, 1.0, bk_sb, "k", first_on_pool=True)
                for i in range(DT):
                    nc.sync.dma_start(wv_sb[i][:], wvT[i * 128:(i + 1) * 128, :])
                for i in range(DT):
                    nc.sync.dma_start(wq_sb[i][:], wqT[i * 128:(i + 1) * 128, :])

                def qproj_b(qc, xp, xtag, psp, pstag):
                    qt_t = pp.tile([128, DH // 128, QCS], F32R, tag="qt",
                                   name="qt_t", bufs=2)
                    xq2 = [xp.tile([128, DT // 2, QCS], F32R, tag=xtag, name="xq2",
                                   bufs=2) for _ in range(2)]
                    for c in range(2):
                        xs = xqT[c * (D // 2):(c + 1) * (D // 2),
                                 qc * QCS:(qc + 1) * QCS].rearrange(
                            "(t p) s -> p t s", p=128)
                        nc.sync.dma_start(xq2[c][:], xs)
                    for dt in range(DH // 128):
                        ps = psp.tile([128, QCS], F32, tag=pstag, name="ps_q")
                        for i in range(DT):
                            nc.tensor.matmul(
                                ps[:], wq_sb[i][:, dt * 128:(dt + 1) * 128],
                                xq2[i // (DT // 2)][:, i % (DT // 2), :],
                                start=(i == 0), stop=(i == DT - 1))
                        nc.vector.tensor_scalar(
                            qt_t[:, dt, :], ps[:], 0.125, bq_sb[:, dt:dt + 1],
                            mybir.AluOpType.mult, mybir.AluOpType.add)
                    qt_tiles[qc] = qt_t

                qt_tiles = {}

                for st4 in range(KT // 4):
                    xv4 = xpool.tile([128, DT, QCS], F32R, tag="x", name="xv4", bufs=2)
                    nc.gpsimd.dma_start(
                        xv4[:],
                        xvT[:, st4 * QCS:(st4 + 1) * QCS].rearrange(
                            "(t p) s -> p t s", p=128))
                    for sub in range(4):
                        st = st4 * 4 + sub
                        ps = pspool.tile([128, DH], F32, tag="ps", name="ps_v")
                        for i in range(DT):
                            nc.tensor.matmul(
                                ps[:], xv4[:, i, sub * 128:(sub + 1) * 128],
                                wv_sb[i][:], start=(i == 0), stop=(i == DT - 1))
                        vdst = v_sb[st].rearrange("p (h c) -> p h c", h=HPG)[:, :, 0:64]
                        nc.vector.tensor_copy(
                            vdst, ps[:].rearrange("p (h c) -> p h c", h=HPG))
                qproj_b(0, xpool, "x", pspool, "ps")

            # wo loads can land any time before the first output projection
            for i in range(DH // 128):
                nc.gpsimd.dma_start(wo_sb[i][:], woT[i * 128:(i + 1) * 128, :])

            # ---- phase B: attention + pipelined output projection ----
            with (
                tc.tile_pool(name="work", bufs=2) as wp,
                tc.tile_pool(name="psS", bufs=2, space="PSUM") as psS,
                tc.tile_pool(name="psC", bufs=4, space="PSUM") as psC,
            ):
                prev = None  # (cpair tiles, qc) pending output projection

                def emit_outproj(cpair, qc, ots, tail=False):
                    for ot in ots:
                        po = psC.tile([128, QCS], F32, tag="cps", name="po")
                        for j in range(HPG // 2):
                            nc.tensor.matmul(
                                po[:], wo_sb[j][:, ot * 128:(ot + 1) * 128],
                                cpair[j][:],
                                start=(j == 0), stop=(j == HPG // 2 - 1))
                        o_sb = wp.tile([128, QCS], F32, tag="o", name="o_sb", bufs=3)
                        if tail and ot % 2 == 0:
                            nc.scalar.activation(
                                o_sb[:], po[:], mybir.ActivationFunctionType.Identity,
                                bias=vr_sb[:, ot:ot + 1])
                        else:
                            nc.vector.tensor_scalar(
                                o_sb[:], po[:], 1.0, vr_sb[:, ot:ot + 1],
                                mybir.AluOpType.mult, mybir.AluOpType.add)
                        eng = nc.gpsimd if ot % 2 else nc.sync
                        eng.dma_start(
                            outT[ot * 128:(ot + 1) * 128, qc * QCS:(qc + 1) * QCS],
                            o_sb[:])

                def load_mask(qc):
                    mask_sb = wp.tile([128, KT, QCS], BF16, tag="mask", name="mask_sb",
                                      bufs=2)
                    ms = maskT[:, qc * QCS:(qc + 1) * QCS].rearrange(
                        "(t p) s -> p t s", p=128)
                    hm = KT // 2
                    nc.sync.dma_start(mask_sb[:, 0:hm, :], ms[:, 0:hm, :])
                    nc.gpsimd.dma_start(mask_sb[:, hm:KT, :], ms[:, hm:KT, :])
                    return mask_sb

                mask_next = load_mask(0)
                for qc in range(QCN):
                    mask_sb = mask_next
                    qt_cur = qt_tiles.pop(qc)
                    cpair_t = wp.tile([128, HPG // 2, QCS], F32R, tag="cp",
                                      name="cpair_t", bufs=2)
                    cpair = [cpair_t[:, j, :] for j in range(HPG // 2)]
                    for j in range(HPG // 2):
                        dtile = j
                        cps = [psC.tile([128, QCS], F32, tag="cps", name=f"cps{hh}",
                                        bufs=4) for hh in range(2)]
                        for kt in range(KT):
                            sps = psS.tile([128, 2, QCS], F32, tag="sps", name="sps",
                                           bufs=2)
                            for hh in range(2):
                                prow = hh * 64
                                nc.tensor.matmul(
                                    sps[:, hh, :],
                                    kt_sb[dtile][prow:prow + 64, kt * 128:(kt + 1) * 128],
                                    qt_cur[prow:prow + 64, dtile, :],
                                    start=True, stop=True)
                            e_sb = wp.tile([128, 2, QCS], BF16, tag="e", name="e_sb",
                                           bufs=4)
                            nc.scalar.activation(
                                e_sb[:], sps[:],
                                mybir.ActivationFunctionType.Exp)
                            meng = nc.gpsimd if kt % 4 == 3 else nc.vector
                            for hh in range(2):
                                meng.tensor_mul(
                                    e_sb[:, hh, :], e_sb[:, hh, :], mask_sb[:, kt, :])
                            for hh in range(2):
                                h = 2 * j + hh
                                nc.tensor.matmul(
                                    cps[hh][0:65, :],
                                    v_sb[kt][:, h * 65:(h + 1) * 65],
                                    e_sb[:, hh, :],
                                    start=(kt == 0), stop=(kt == KT - 1))
                        for hh in (1, 0):
                            # normalize: C[d, q] / Z[q]; Z = PSUM row 64.
                            # The scalar engine does a partition-SHIFTING copy
                            # (PSUM row 64 -> SBUF row 0), reciprocal runs at
                            # partition 0 (custom-DVE PSUM reads at partition
                            # 64 are broken on HW), then broadcast + multiply.
                            ns = wp.tile([64, 2, QCS], F32, tag="ns", name="ns", bufs=2)
                            nc.scalar.copy(ns[0:1, 0, :], cps[hh][64:65, :])
                            nc.vector.reciprocal_approx_fast(
                                out=ns[0:1, 1, :], in_=ns[0:1, 0, :])
                            rb = ns[0:64, 0, :]
                            nc.gpsimd.partition_broadcast(rb, ns[0:1, 1, :],
                                                          channels=64)
                            if hh == 0:
                                nc.vector.tensor_mul(cpair[j][0:64, :],
                                                     cps[hh][0:64, :], rb)
                            else:
                                cstage = wp.tile([64, QCS], F32R, tag="cstage",
                                                 name="cstage", bufs=3)
                                nc.vector.tensor_mul(cstage[:], cps[hh][0:64, :], rb)
                                nc.sync.dma_start(cpair[j][64:128, :], cstage[:])
                        if prev is not None:
                            emit_outproj(prev[0], prev[1], range(2 * j, 2 * j + 2))
                        if j == 0 and qc + 1 < QCN:
                            mask_next = load_mask(qc + 1)
                        if j == 1 and qc + 1 < QCN:
                            qproj_b(qc + 1, wp, "xq", psC, "cps")
                    prev = (cpair, qc)
                # drain the last q chunk's output projection
                emit_outproj(prev[0], prev[1], range(D // 128), tail=True)

    nc.finalize()
    return nc


_NC_CACHE = None


def _get_nc():
    global _NC_CACHE
    if _NC_CACHE is None:
        _NC_CACHE = build_nc()
    return _NC_CACHE


def shard_inputs(query, key, value, mask, wq, bq, wk, bk, wv, bv, wo, bo):
    """Build the per-core input maps (host-side shard prep)."""
    import ml_dtypes

    query = np.asarray(query, np.float32)
    key = np.asarray(key, np.float32)
    value = np.asarray(value, np.float32)
    mask = np.asarray(mask)
    wq = np.asarray(wq, np.float32); bq = np.asarray(bq, np.float32)
    wk = np.asarray(wk, np.float32); bk = np.asarray(bk, np.float32)
    wv = np.asarray(wv, np.float32); bv = np.asarray(bv, np.float32)
    wo = np.asarray(wo, np.float32); bo = np.asarray(bo, np.float32)

    in_maps = []
    maskT_b = [np.ascontiguousarray(mask[b].T).astype(ml_dtypes.bfloat16)
               for b in range(B)]
    xT = {}
    for b in range(B):
        xT[b] = (
            np.ascontiguousarray(query[b].T),
            np.ascontiguousarray(key[b].T),
            np.ascontiguousarray(value[b].T),
        )
    for c in range(NCORES):
        b, hg = divmod(c, HG)
        sl = slice(hg * DH, (hg + 1) * DH)
        wo_block = wo[:, sl]                       # [1024, 512]
        v_r = bv[sl] @ wo_block.T                  # [1024]
        if hg == 0:
            v_r = v_r + bo
        in_maps.append({
            "xqT": xT[b][0],
            "xkT": xT[b][1],
            "xvT": xT[b][2],
            "maskT": maskT_b[b],
            "wqT": np.ascontiguousarray(wq[sl].T),
            "wkT": np.ascontiguousarray(wk[sl].T),
            "wvT": np.ascontiguousarray(wv[sl].T),
            "woT": np.ascontiguousarray(wo_block.T),
            "bq2": np.ascontiguousarray((bq[sl] / 8.0).reshape(DH // 128, 128).T),
            "bk2": np.ascontiguousarray(bk[sl].reshape(DH // 128, 128).T),
            "vr2": np.ascontiguousarray(v_r.reshape(D // 128, 128).T),
        })
    return in_maps


def combine_outputs(results):
    """results: list of per-core {"outT": [1024, 2048]} -> full [B, S, D]."""
    out = np.empty((B, S, D), np.float32)
    for b in range(B):
        acc = results[2 * b]["outT"] + results[2 * b + 1]["outT"]
        out[b] = acc.T
    return out


def kernel(**inputs):
    from concourse.bass_utils import run_bass_kernel_spmd

    nc = _get_nc()
    in_maps = shard_inputs(**inputs)
    res = run_bass_kernel_spmd(nc, in_maps, list(range(NCORES)))
    return combine_outputs(res.results)



# revision 64
# speedup vs baseline: 1.2625x; 1.2625x over previous
"""Multi-head attention Trainium2 Bass kernel, sharded over 8 NeuronCores.

Problem: B=4, S=2048, D=1024, H=16 heads (DK=64), fp32, random 0/1 mask.

Sharding (data-parallel batch x tensor-parallel heads):
  core c handles batch b = c // 2, head-group hg = c % 2 (8 heads = 512 dims).
  Host sums the two head-group partials per batch.

Design (v2, fp8 DoubleRow):
  The attention inner loop is Activation-engine bound (exp of every score,
  256 x [128,1024] ACTIVATE = ~266us floor). Everything else is scheduled to
  hide under it:
  - Q/K/V projections run as fp8e4 DoubleRow matmuls (hi/lo split of both
    inputs and weights, 3 products, weights pre-scaled x16 on host) at 4x
    the f32r rate, so the serial phase-A head shrinks to ~7us and V/Q
    projections stream inside the attention windows.
  - Scores are fp8 DoubleRow: Q^T/K^T are quantized to fp8 by the projection
    epilogue into a [32p, 2slot, n] layout (d split 32+32), one 256-cycle
    matmul per (head, kt). The 1/sqrt(dk) lands in the exp's scale operand.
  - P@V is restructured: E tiles ([k,q] bf16) are the stationary operand and
    V ([k,64] bf16) moves, so each kt costs 65 cols instead of 512. Output
    C is [q, d] per (head, qsub); the softmax sum Z rides as 1-col matmuls
    into a shared PSUM bank (single start=True per bank, pending-zero
    zero-fill for the other accumulators).
  - Normalization becomes per-partition: reciprocal(Z) + tensor_scalar
    multiply -> bf16, then a PE transpose (identity moving) rebuilds the
    [d, q] cpair layout for the output projection (bf16 weights).
  - The whole C/Z phase for sweep (qc, j) is emitted one j-sweep behind the
    scores/exp/mask of (qc, j+1), so kt-boundary dependencies never stall
    the in-order queues; V projection chunks and the output projection of
    the previous qc fill the leftover PE slack.

PSUM: sps 2x2 banks, C 2x1, Z+ct 1 (f32 Z cols + bf16 transpose staging via
bitcast), proj/outproj ring 1 = 8 banks.
"""
import numpy as np

import concourse.bass as bass
import concourse.mybir as mybir
import concourse.tile as tile
from concourse import bacc

B, S, D, H = 4, 2048, 1024, 16
DK = D // H          # 64
NCORES = 8
HG = 2               # head groups (tensor-parallel factor per batch)
HPG = H // HG        # 8 heads per core
DH = D // HG         # 512 head dims per core
QCN = 4              # q chunks
QCS = S // QCN       # 512
KT = S // 128        # 16 k tiles
F32 = mybir.dt.float32
F8 = mybir.dt.float8e4
BF16 = mybir.dt.bfloat16
DR = mybir.MatmulPerfMode.DoubleRow


def build_nc():
    nc = bacc.Bacc(None)
    # x inputs: [128, 4 step, 2 slot, S] fp8 (din = step*256+slot*128+p);
    # xq/xk ship only the hi plane (2-product projection), xv hi+lo
    xq8 = [nc.declare_dram_parameter("xq80", [128, 4, 2, S], F8, isOutput=False)]
    xk8 = [nc.declare_dram_parameter("xk80", [128, 4, 2, S], F8, isOutput=False)]
    xv8 = [nc.declare_dram_parameter(f"xv8{t}", [128, 4, 2, S], F8, isOutput=False)
           for t in range(2)]
    # weights: [128, 4 step, 2 slot, 512 outcol] hi/lo fp8, x16, out-permuted
    wq8 = [nc.declare_dram_parameter(f"wq8{t}", [128, 4, 2, DH], F8, isOutput=False)
           for t in range(2)]
    wk8 = [nc.declare_dram_parameter(f"wk8{t}", [128, 4, 2, DH], F8, isOutput=False)
           for t in range(2)]
    wv8 = [nc.declare_dram_parameter(f"wv8{t}", [128, 4, 2, DH], F8, isOutput=False)
           for t in range(2)]
    maskT = nc.declare_dram_parameter("maskT", [S, S], BF16, isOutput=False)
    woT = nc.declare_dram_parameter("woT", [DH, D], BF16, isOutput=False)
    # bqk: cols 0-3 = bq' per (s,half) tile, cols 4-7 = bk'
    bqk = nc.declare_dram_parameter("bqk", [128, 8], F32, isOutput=False)
    vr2 = nc.declare_dram_parameter("vr2", [128, D // 128], F32, isOutput=False)
    ident = nc.declare_dram_parameter("ident", [128, 128], BF16, isOutput=False)
    outT = nc.declare_dram_parameter("outT", [D, S], BF16, isOutput=True)

    with tile.TileContext(nc) as tc:
        with (
            tc.tile_pool(name="persist", bufs=1) as pp,
            tc.tile_pool(name="work", bufs=2) as wp,
            tc.tile_pool(name="psS", bufs=2, space="PSUM") as psS,
            tc.tile_pool(name="psC", bufs=2, space="PSUM") as psC,
            tc.tile_pool(name="psZ", bufs=1, space="PSUM") as psZ,
            tc.tile_pool(name="psO", bufs=1, space="PSUM") as psO,
        ):
            # ---------------- persistent tiles ----------------
            # K^T fp8 per sweep s: [128 (m*32+d32), 2 half, S]
            k8 = [pp.tile([128, 2, S], F8, tag=f"k8_{s}", name=f"k8_{s}")
                  for s in range(2)]
            # V: [128 k, kt, 8h*65] bf16 (64 vals + ones col per head)
            v_full = pp.tile([128, KT, HPG * 65], BF16, tag="v", name="v_full")
            v_sb = [v_full[:, i, :] for i in range(KT)]
            wo_sb = pp.tile([128, 4, D], BF16, tag="wo", name="wo_sb")
            bias_sb = pp.tile([128, 16], F32, tag="bias", name="bias_sb")
            bq_sb = bias_sb[:, 0:4]
            bk_sb = bias_sb[:, 4:8]
            vr_sb = bias_sb[:, 8:16]
            id_sb = pp.tile([128, 128], BF16, tag="ident", name="id_sb")
            wq_sb = [pp.tile([128, 4, 2, DH], F8, tag=f"wq{t}", name=f"wq_sb{t}")
                     for t in range(2)]
            wk_sb = [pp.tile([128, 4, 2, DH], F8, tag=f"wk{t}", name=f"wk_sb{t}")
                     for t in range(2)]
            wv_sb = [pp.tile([128, 4, 2, DH], F8, tag=f"wv{t}", name=f"wv_sb{t}")
                     for t in range(2)]

            # ---------------- boot DMAs ----------------
            # PE p-state warmup on a memset tile: no DMA dependency, so the
            # PE busy period starts immediately and the ramp completes
            # before the first real projection tile
            warm_in = pp.tile([128, 128], BF16, tag="warmin", name="warm_in")
            nc.gpsimd.memset(warm_in[:], 0.25)
            warm_ps = psO.tile([128, QCS], F32, tag="po", name="warm_ps")
            for _ in range(12):
                nc.tensor.matmul(
                    warm_ps[:, 0:128], warm_in[:], warm_in[:],
                    start=True, stop=True, skip_group_check=True)
            # wk planes split across both queues so the first K tile can
            # start as early as possible
            nc.sync.dma_start(wk_sb[0][:], wk8[0][:])
            nc.gpsimd.dma_start(wk_sb[1][:], wk8[1][:])
            nc.gpsimd.dma_start(bias_sb[:, 0:8], bqk[:])
            # ones columns of v_full
            ones_view = v_full.rearrange("p t (h c) -> p t h c", h=HPG)[:, :, :, 64:65]
            nc.gpsimd.memset(ones_view, 1.0)
            # preload the Exp activation table off the critical path
            warm_sb = pp.tile([128, 4], F32, tag="warm", name="warm_sb")
            nc.scalar.activation(
                warm_sb[0:1, 0:1], bias_sb[0:1, 0:1],
                mybir.ActivationFunctionType.Exp)

            # chunk-ring staging for the fp8 x streams
            xk_chunks, xq_chunks, xv_chunks = {}, {}, {}
            XBUFS = {"xk": 3, "xq": 2, "xv": 2}

            def xdma(eng, chunks, src, c, tag):
                pair = tuple(
                    wp.tile([128, 4, 2, QCS], F8, tag=f"{tag}{t}",
                            name=f"{tag}_t{t}", bufs=XBUFS[tag])
                    for t in range(len(src)))
                cols = slice(c * QCS, (c + 1) * QCS)
                for t in range(len(src)):
                    eng.dma_start(pair[t][:], src[t][:, :, :, cols])
                chunks[c] = pair

            def dr_prods(ps, wsb, xpair, prods):
                """DoubleRow matmuls: 4 steps x the given (w,x) plane
                products, accumulating stationary w x moving x into ps."""
                first = True
                n = 0
                total = 4 * len(prods)
                for i in range(4):
                    for (tw, tx) in prods:
                        n += 1
                        nc.tensor.matmul(
                            ps[:], wsb[tw][:, i, :, :],
                            xpair[tx][:, i, :, :],
                            start=first, stop=(n == total), perf_mode=DR)
                        first = False

            # per-(s,half) psum tile from the shared psO ring
            def proj_tile(tag="po"):
                return psO.tile([128, QCS], F32, tag=tag, name="proj_ps")

            QK_PRODS = ((0, 0), (1, 0))   # (w_hi + w_lo) x x_hi
            V_PRODS = ((0, 0), (0, 1), (1, 0))
            _tsp_alt = [0]

            def tsp_eng():
                """alternate projection epilogues between DVE and Pool so
                neither becomes the convoy for dependent scores"""
                return nc.vector

            def emit_kproj_tile(c, s, half, pool_tsp=False):
                """K projection, one (s,half) out tile of k-chunk c."""
                cols = slice(c * QCS, (c + 1) * QCS)
                t = s * 2 + half
                ps = proj_tile()
                dr_prods(ps,
                         [wk_sb[0][:, :, :, t * 128:(t + 1) * 128],
                          wk_sb[1][:, :, :, t * 128:(t + 1) * 128]],
                         xk_chunks[c], QK_PRODS)
                eng = nc.gpsimd if pool_tsp else nc.vector
                eng.tensor_scalar(
                    k8[s][:, half, cols], ps[:], 1.0 / 16.0,
                    bk_sb[:, t:t + 1],
                    mybir.AluOpType.mult, mybir.AluOpType.add)

            q8_tiles = {}
            qproj_done = {}

            def emit_qproj_tile(qc, s, half):
                qproj_done[qc] = qproj_done.get(qc, 0) + 1
                if qc not in q8_tiles:
                    q8_tiles[qc] = wp.tile([128, 2, 2, QCS], F8, tag="q8",
                                           name="q8_t", bufs=2)
                q8_t = q8_tiles[qc]
                t = s * 2 + half
                ps = proj_tile()
                dr_prods(ps,
                         [wq_sb[0][:, :, :, t * 128:(t + 1) * 128],
                          wq_sb[1][:, :, :, t * 128:(t + 1) * 128]],
                         xq_chunks[qc], QK_PRODS)
                tsp_eng().tensor_scalar(
                    q8_t[:, s, half, :], ps[:], 1.0 / 16.0,
                    bq_sb[:, t:t + 1],
                    mybir.AluOpType.mult, mybir.AluOpType.add)

            def emit_vproj_tile(ksub):
                """V projection for one 128-k subtile: out [128, 512 feat]."""
                xv_pair = xv_chunks[ksub // 4]
                kcols = slice((ksub % 4) * 128, (ksub % 4 + 1) * 128)
                ps = proj_tile()
                first = True
                n = 0
                for i in range(4):
                    for (tw, tx) in V_PRODS:
                        n += 1
                        nc.tensor.matmul(
                            ps[:], xv_pair[tx][:, i, :, kcols],
                            wv_sb[tw][:, i, :, :],
                            start=first, stop=(n == 12), perf_mode=DR)
                        first = False
                vdst = v_sb[ksub].rearrange(
                    "p (h c) -> p h c", h=HPG)[:, :, 0:64]
                nc.vector.tensor_scalar(
                    vdst, ps[:].rearrange("p (h c) -> p h c", h=HPG),
                    1.0 / 16.0, 0.0,
                    mybir.AluOpType.mult, mybir.AluOpType.add)

            def load_mask(qc):
                mask_sb = wp.tile([128, KT, QCS], BF16, tag="mask",
                                  name="mask_sb", bufs=2)
                ms = maskT[:, qc * QCS:(qc + 1) * QCS].rearrange(
                    "(t p) s -> p t s", p=128)
                hm = KT // 2
                nc.sync.dma_start(mask_sb[:, 0:hm, :], ms[:, 0:hm, :])
                nc.gpsimd.dma_start(mask_sb[:, hm:KT, :], ms[:, hm:KT, :])
                return mask_sb

            # ---------------- phase A: minimal head ----------------
            xdma(nc.sync, xk_chunks, xk8, 0, "xk")
            xdma(nc.gpsimd, xq_chunks, xq8, 0, "xq")
            for t in range(2):
                nc.gpsimd.dma_start(wq_sb[t][:], wq8[t][:])
            nc.gpsimd.dma_start(id_sb[:], ident[:])
            nc.gpsimd.dma_start(vr_sb[:, :], vr2[:])
            # head: only the sweep-0 tiles attention j0 needs immediately
            emit_kproj_tile(0, 0, 0)
            emit_kproj_tile(0, 0, 1)
            emit_qproj_tile(0, 0, 0)
            emit_qproj_tile(0, 0, 1)
            mask0 = load_mask(0)
            xdma(nc.sync, xk_chunks, xk8, 1, "xk")
            xdma(nc.sync, xk_chunks, xk8, 2, "xk")
            for t in range(2):
                nc.gpsimd.dma_start(wv_sb[t][:], wv8[t][:])
            xdma(nc.gpsimd, xv_chunks, xv8, 0, "xv")
            xdma(nc.sync, xv_chunks, xv8, 1, "xv")
            nc.gpsimd.dma_start(
                wo_sb[:],
                woT.rearrange("(j p) d -> p j d", p=128))

            # ---------------- phase B ----------------
            # sweep state carried between windows
            sweeps = {}   # (qc, j) -> dict(e=..list of e tiles.., ..)
            cpairs = {}   # (qc, j) -> cpair AP

            def emit_scores_exp_mask(qc, j, kt, mask_sb, q8_t):
                sps = psS.tile([128, 2, QCS], F32, tag="sps", name="sps", bufs=2)
                for hh in range(2):
                    h = 2 * j + hh
                    s, m = h // 4, h % 4
                    nc.tensor.matmul(
                        sps[:, hh, :],
                        k8[s][32 * m:32 * m + 32, :, kt * 128:(kt + 1) * 128],
                        q8_t[32 * m:32 * m + 32, s, :, :],
                        start=True, stop=True, perf_mode=DR,
                        tile_position=(32 * m, 0))
                e_sb = wp.tile([128, 2, QCS], BF16, tag="e", name="e_sb", bufs=30)
                nc.scalar.activation(
                    e_sb[:], sps[:], mybir.ActivationFunctionType.Exp,
                    scale=0.125)
                meng = nc.vector if kt % 2 == 0 else nc.gpsimd
                for hh in range(2):
                    meng.tensor_mul(e_sb[:, hh, :], e_sb[:, hh, :],
                                    mask_sb[:, kt, :])
                return e_sb

            # single persistent Z bank; sweeps alternate 8-col groups
            z_bank = psZ.tile([128, QCS], F32, tag="z", name="z_bank")

            def start_sweep(qc, j):
                n = 4 * qc + j
                cps = psC.tile([128, QCS], F32, tag="c", name="cps", bufs=2)
                zoff = 256 + 8 * (n % 2)
                return {"qc": qc, "j": j, "c": cps, "z": z_bank, "zoff": zoff,
                        "n": 0}

            def emit_cz(sw, e_sb, kt):
                """C and Z matmuls for one kt of the lagged sweep."""
                j, cps = sw["j"], sw["c"]
                zb, zoff = sw["z"], sw["zoff"]
                for hh in range(2):
                    h = 2 * j + hh
                    for qsub in range(4):
                        idx = hh * 4 + qsub
                        est = e_sb[:, hh, qsub * 128:(qsub + 1) * 128]
                        first = (sw["n"] == 0)
                        sw["n"] += 1
                        nc.tensor.matmul(
                            cps[:, idx * 64:(idx + 1) * 64], est,
                            v_sb[kt][:, h * 65:h * 65 + 64],
                            start=first, stop=(kt == KT - 1 and idx == 7),
                            skip_group_check=True)
                        nc.tensor.matmul(
                            zb[:, zoff + idx:zoff + idx + 1], est,
                            v_sb[kt][:, h * 65 + 64:h * 65 + 65],
                            start=False, stop=(kt == KT - 1 and idx == 7),
                            skip_group_check=True)

            def emit_sweep_epilogue(sw):
                """recip Z, normalize -> bf16, PE transpose (into the C bank
                after its stripes are consumed), cpair copy."""
                qc, j, cps = sw["qc"], sw["j"], sw["c"]
                zb, zoff = sw["z"], sw["zoff"]
                rz = wp.tile([128, 8], F32, tag="rz", name="rz", bufs=2)
                nc.vector.reciprocal_approx_fast(
                    out=rz[:], in_=zb[:, zoff:zoff + 8])
                cn = wp.tile([128, 8, 64], BF16, tag="cn", name="cn", bufs=2)
                for idx in range(8):
                    nc.vector.tensor_scalar(
                        cn[:, idx, :], cps[:, idx * 64:(idx + 1) * 64],
                        rz[:, idx:idx + 1], 0.0,
                        mybir.AluOpType.mult, mybir.AluOpType.add)
                # transposes stage through a transient psO-ring tile: the
                # first one's start=True zeroes the bank, the rest rely on
                # the pending-zero it leaves (same pattern as Z/C banks)
                ct = psO.tile([128, QCS], BF16, tag="po", name="ct")
                first = True
                for hh in range(2):
                    for qsub in range(4):
                        idx = hh * 4 + qsub
                        nc.tensor.matmul(
                            ct[hh * 64:hh * 64 + 64,
                               qsub * 128:(qsub + 1) * 128],
                            cn[:, idx, :], id_sb[:],
                            is_transpose=True, start=first, stop=True,
                            skip_group_check=True)
                        first = False
                cpair = wp.tile([128, QCS], BF16, tag="cpair", name="cpair",
                                bufs=5)
                nc.vector.tensor_copy(cpair[:], ct[:])
                cpairs[(qc, j)] = cpair

            def emit_outproj_ot(qc, ot):
                cp = [cpairs[(qc, j)] for j in range(4)]
                po = psO.tile([128, QCS], F32, tag="po", name="po")
                for j in range(4):
                    nc.tensor.matmul(
                        po[:], wo_sb[:, j, ot * 128:(ot + 1) * 128],
                        cp[j][:], start=(j == 0), stop=(j == 3))
                o_sb = wp.tile([128, QCS], BF16, tag="o", name="o_sb", bufs=3)
                nc.vector.tensor_scalar(
                    o_sb[:], po[:], 1.0, vr_sb[:, ot:ot + 1],
                    mybir.AluOpType.mult, mybir.AluOpType.add)
                eng = nc.gpsimd if ot % 2 else nc.sync
                eng.dma_start(
                    outT[ot * 128:(ot + 1) * 128, qc * QCS:(qc + 1) * QCS],
                    o_sb[:])

            # ---- budgeted PE work queue ----
            # Items: (cost_ns, thunk). Emitted in FIFO order, paced so each
            # slot adds at most ~BUDGET ns of PE work on top of the scores.
            from collections import deque
            work = deque()
            BUDGET = 800.0
            CAP = 2000.0
            allowance = [0.0]

            def kt_(f, *a, **kw):
                return lambda: f(*a, **kw)

            def push(cost, thunk):
                work.append((cost, thunk))

            def drain_slot():
                allowance[0] = min(allowance[0] + BUDGET, CAP)
                while work and allowance[0] >= work[0][0]:
                    cost, thunk = work.popleft()
                    allowance[0] -= cost
                    thunk()

            def drain_all():
                while work:
                    _, thunk = work.popleft()
                    thunk()

            # fixed j0 slot tasks: the K tiles attention depends on (hard
            # deadlines), chunk-paced with their ring DMAs. Fixed slot k
            # fires right after scores(k+1) is emitted; the sweep-0 half of
            # chunk c must land before scores(kt=4c).
            fixed = {}
            fixed[(0, 0, 0)] = [kt_(emit_kproj_tile, 1, 0, 0)]
            fixed[(0, 0, 1)] = [kt_(emit_kproj_tile, 1, 0, 1)]
            fixed[(0, 0, 2)] = [kt_(emit_kproj_tile, 0, 1, 0)]
            fixed[(0, 0, 3)] = [kt_(emit_kproj_tile, 0, 1, 1)]
            fixed[(0, 0, 4)] = [kt_(xdma, nc.sync, xk_chunks, xk8, 3, "xk"),
                                kt_(emit_kproj_tile, 2, 0, 0)]
            fixed[(0, 0, 5)] = [kt_(emit_kproj_tile, 2, 0, 1)]
            fixed[(0, 0, 6)] = [kt_(emit_kproj_tile, 1, 1, 0)]
            fixed[(0, 0, 7)] = [kt_(emit_kproj_tile, 1, 1, 1)]
            fixed[(0, 0, 8)] = [kt_(emit_kproj_tile, 3, 0, 0)]
            fixed[(0, 0, 9)] = [kt_(emit_kproj_tile, 3, 0, 1)]
            fixed[(0, 0, 10)] = [kt_(emit_kproj_tile, 2, 1, 0)]
            fixed[(0, 0, 11)] = [kt_(emit_kproj_tile, 2, 1, 1)]
            fixed[(0, 0, 12)] = [kt_(emit_kproj_tile, 3, 1, 0)]
            fixed[(0, 0, 13)] = [kt_(emit_kproj_tile, 3, 1, 1)]
            fixed[(0, 0, 14)] = [kt_(emit_qproj_tile, 0, 1, 0)]
            fixed[(0, 0, 15)] = [kt_(emit_qproj_tile, 0, 1, 1)]
            for qcn in range(1, QCN):
                eng_x = nc.gpsimd if qcn % 2 else nc.sync
                fixed[(qcn - 1, 1, 12)] = [
                    kt_(xdma, eng_x, xq_chunks, xq8, qcn, "xq")]

            TILE_NS = 1300.0
            CZ_NS = 230.0
            EPI_NS = 900.0
            OT_NS = 880.0

            epi_count = [0]
            oa_tiles = {}

            def emit_outproj_partial(ot):
                """qc3 ots, j0-j2 partial accumulated early -> bf16 SBUF"""
                cp = [cpairs[(QCN - 1, j)] for j in range(3)]
                po = psO.tile([128, QCS], F32, tag="po", name="po")
                for j in range(3):
                    nc.tensor.matmul(
                        po[:], wo_sb[:, j, ot * 128:(ot + 1) * 128],
                        cp[j][:], start=(j == 0), stop=(j == 2))
                oa = wp.tile([128, QCS], BF16, tag="oa", name="oa", bufs=8)
                nc.vector.tensor_scalar(
                    oa[:], po[:], 1.0, 0.0,
                    mybir.AluOpType.mult, mybir.AluOpType.add)
                oa_tiles[ot] = oa

            def emit_epi(sw):
                emit_sweep_epilogue(sw)
                epi_count[0] += 1
                qc, j = sw["qc"], sw["j"]
                if j == 3 and qc < QCN - 1:
                    for ot in range(8):
                        push(OT_NS, kt_(emit_outproj_ot, qc, ot))
                if qc == QCN - 1 and j == 2:
                    for ot in range(8):
                        push(OT_NS, kt_(emit_outproj_partial, ot))

            def push_sweep(sw):
                """queue the CZ phase + epilogue of a finished sweep
                (early sweeps only; later sweeps inline their CZ)."""
                qc, j = sw["qc"], sw["j"]
                zb, zoff = sw["z"], sw["zoff"]
                # zero this sweep's Z col-group (queued, so it lands after
                # the PREVIOUS same-group sweep's Z reads)
                push(0, kt_(nc.vector.memset, zb[:, zoff:zoff + 8], 0.0))
                if qc == 0 and j == 0:
                    # V tiles interleave just ahead of the CZ kts that
                    # consume them (V projection happens here, in the lag
                    # window, not in phase A)
                    for kt in range(KT):
                        push(TILE_NS, kt_(emit_vproj_tile, kt))
                        push(CZ_NS, kt_(emit_cz, sw, sw["e"][kt], kt))
                        if kt == 3:
                            push(0, kt_(xdma, nc.gpsimd, xv_chunks, xv8,
                                        2, "xv"))
                        if kt == 7:
                            push(0, kt_(xdma, nc.sync, xv_chunks, xv8,
                                        3, "xv"))
                else:
                    for kt in range(KT):
                        push(CZ_NS, kt_(emit_cz, sw, sw["e"][kt], kt))
                push(EPI_NS, kt_(emit_epi, sw))

            INLINE_FROM = 4   # sweeps with index >= this inline their CZ
            LAG = 3
            mask_next = mask0
            for qc in range(QCN):
                mask_sb = mask_next
                if qc > 0:
                    # this qc's Q tiles must be emitted before its scores
                    while qproj_done.get(qc, 0) < 4:
                        _, thunk = work.popleft()
                        thunk()
                for j in range(4):
                    n = 4 * qc + j
                    q8_t = q8_tiles[qc]
                    sw = start_sweep(qc, j)
                    inline = (n >= INLINE_FROM)
                    if inline:
                        # predecessors of this sweep's C bank and Z group
                        # must be fully consumed before we touch them
                        while epi_count[0] < n - 1:
                            _, thunk = work.popleft()
                            thunk()
                        nc.vector.memset(
                            z_bank[:, sw["zoff"]:sw["zoff"] + 8], 0.0)
                    for kt in range(KT):
                        e_sb = emit_scores_exp_mask(qc, j, kt, mask_sb, q8_t)
                        sw.setdefault("e", []).append(e_sb)
                        # lookahead: scores/exp/mask of kt are emitted before
                        # the budgeted work of kt-1's slot
                        if kt > 0:
                            for thunk in fixed.pop((qc, j, kt - 1), ()):
                                thunk()
                            drain_slot()
                        if inline and kt >= LAG:
                            emit_cz(sw, sw["e"][kt - LAG], kt - LAG)
                        if j == 3 and kt == 3 and qc + 1 < QCN:
                            mask_next = load_mask(qc + 1)
                        if j == 2 and kt in (2, 5, 8, 11) and qc + 1 < QCN:
                            push(TILE_NS, kt_(emit_qproj_tile, qc + 1,
                                              (kt - 2) // 6, ((kt - 2) // 3) % 2))

                    for thunk in fixed.pop((qc, j, KT - 1), ()):
                        thunk()
                    drain_slot()
                    if inline:
                        for kt in range(KT - LAG, KT):
                            emit_cz(sw, sw["e"][kt], kt)
                        emit_epi(sw)
                    else:
                        push_sweep(sw)
                q8_tiles.pop(qc)
            drain_all()
            # final output projection (qc3): j3 delta on top of staged
            # j0-j2 partials, pipelined through both free PSUM rings
            for ot in range(8):
                if ot % 2:
                    po = psO.tile([128, QCS], F32, tag="po", name="po2")
                else:
                    po2 = psS.tile([128, 2, QCS], F32, tag="sps", name="po2",
                                   bufs=2)
                    po = po2[:, 0, :]
                nc.tensor.matmul(
                    po[:], wo_sb[:, 3, ot * 128:(ot + 1) * 128],
                    cpairs[(QCN - 1, 3)][:], start=True, stop=True)
                o_sb = wp.tile([128, QCS], BF16, tag="o", name="o_sb", bufs=3)
                nc.vector.scalar_tensor_tensor(
                    o_sb[:], po[:], vr_sb[:, ot:ot + 1], oa_tiles[ot][:],
                    mybir.AluOpType.add, mybir.AluOpType.add)
                eng = nc.gpsimd if ot % 2 else nc.sync
                eng.dma_start(
                    outT[ot * 128:(ot + 1) * 128,
                         (QCN - 1) * QCS:QCN * QCS],
                    o_sb[:])

    nc.finalize()
    return nc


_NC_CACHE = None


def _get_nc():
    global _NC_CACHE
    if _NC_CACHE is None:
        _NC_CACHE = build_nc()
    return _NC_CACHE


def _hi_lo_fp8(x):
    import ml_dtypes
    f8 = ml_dtypes.float8_e4m3
    hi = x.astype(f8)
    lo = (x - hi.astype(np.float32)).astype(f8)
    return hi, lo


def _x_prep(xT):
    """[D, S] f32 -> ([128, 4, 2, S] hi, lo) fp8 with din=i*256+s*128+p."""
    r = xT.reshape(4, 2, 128, xT.shape[1]).transpose(2, 0, 1, 3)
    return _hi_lo_fp8(np.ascontiguousarray(r))


def _w_prep(w_slice, perm):
    """w_slice [512 outf, 1024 din] -> ([128, 4, 2, 512] hi, lo) fp8 x16.
    Column c of the output = out-feature perm[c]."""
    w = (16.0 * w_slice[perm]).T          # [1024 din, 512 outcol]
    r = w.reshape(4, 2, 128, 512).transpose(2, 0, 1, 3)
    return _hi_lo_fp8(np.ascontiguousarray(r))


def shard_inputs(query, key, value, mask, wq, bq, wk, bk, wv, bv, wo, bo):
    import ml_dtypes
    bf = ml_dtypes.bfloat16

    query = np.asarray(query, np.float32)
    key = np.asarray(key, np.float32)
    value = np.asarray(value, np.float32)
    mask = np.asarray(mask)
    wq = np.asarray(wq, np.float32); bq = np.asarray(bq, np.float32)
    wk = np.asarray(wk, np.float32); bk = np.asarray(bk, np.float32)
    wv = np.asarray(wv, np.float32); bv = np.asarray(bv, np.float32)
    wo = np.asarray(wo, np.float32); bo = np.asarray(bo, np.float32)

    # out-feature permutation for Q/K: col (t, oc) -> f = (4s+oc//32)*64
    #  + 32*half + oc%32, t = 2s+half
    perm = np.empty(512, np.int64)
    for s_ in range(2):
        for half in range(2):
            t = 2 * s_ + half
            oc = np.arange(128)
            perm[t * 128:(t + 1) * 128] = ((4 * s_ + oc // 32) * 64
                                           + 32 * half + oc % 32)

    ident = np.eye(128, dtype=np.float32).astype(bf)
    maskT_b = [np.ascontiguousarray(mask[b].T).astype(bf) for b in range(B)]
    xp = {}
    for b in range(B):
        xp[b] = (
            _x_prep(np.ascontiguousarray(query[b].T)),
            _x_prep(np.ascontiguousarray(key[b].T)),
            _x_prep(np.ascontiguousarray(value[b].T)),
        )

    in_maps = []
    for c in range(NCORES):
        b, hg = divmod(c, HG)
        sl = slice(hg * DH, (hg + 1) * DH)
        wo_block = wo[:, sl]                       # [1024, 512]
        v_r = bv[sl] @ wo_block.T                  # [1024]
        if hg == 0:
            v_r = v_r + bo
        wq_hl = _w_prep(wq[sl], perm)
        wk_hl = _w_prep(wk[sl], perm)
        wv_hl = _w_prep(wv[sl], np.arange(512))
        bqk_arr = np.zeros((128, 8), np.float32)
        for t in range(4):
            p = np.arange(128)
            f = perm[t * 128 + p]
            bqk_arr[:, t] = bq[sl][f]
            bqk_arr[:, 4 + t] = bk[sl][f]
        (xq_hi, _), (xk_hi, _), (xv_hi, xv_lo) = xp[b]
        in_maps.append({
            "xq80": xq_hi,
            "xk80": xk_hi,
            "xv80": xv_hi, "xv81": xv_lo,
            "wq80": wq_hl[0], "wq81": wq_hl[1],
            "wk80": wk_hl[0], "wk81": wk_hl[1],
            "wv80": wv_hl[0], "wv81": wv_hl[1],
            "maskT": maskT_b[b],
            "woT": np.ascontiguousarray(wo_block.T).astype(bf),
            "bqk": bqk_arr,
            "vr2": np.ascontiguousarray(v_r.reshape(D // 128, 128).T),
            "ident": ident,
        })
    return in_maps


def combine_outputs(results):
    """results: list of per-core {"outT": [1024, 2048] bf16} -> [B, S, D]."""
    out = np.empty((B, S, D), np.float32)
    for b in range(B):
        acc = (results[2 * b]["outT"].astype(np.float32)
               + results[2 * b + 1]["outT"].astype(np.float32))
        out[b] = acc.T
    return out


def kernel(**inputs):
    from concourse.bass_utils import run_bass_kernel_spmd

    nc = _get_nc()
    in_maps = shard_inputs(**inputs)
    res = run_bass_kernel_spmd(nc, in_maps, list(range(NCORES)))
    return combine_outputs(res.results)


# revision 70
# speedup vs baseline: 1.2645x; 1.0016x over previous
"""Multi-head attention Trainium2 Bass kernel, sharded over 8 NeuronCores.

Problem: B=4, S=2048, D=1024, H=16 heads (DK=64), fp32, random 0/1 mask.

Sharding (data-parallel batch x tensor-parallel heads):
  core c handles batch b = c // 2, head-group hg = c % 2 (8 heads = 512 dims).
  Host sums the two head-group partials per batch.

Design (v2, fp8 DoubleRow):
  The attention inner loop is Activation-engine bound (exp of every score,
  256 x [128,1024] ACTIVATE = ~266us floor). Everything else is scheduled to
  hide under it:
  - Q/K/V projections run as fp8e4 DoubleRow matmuls (hi/lo split of both
    inputs and weights, 3 products, weights pre-scaled x16 on host) at 4x
    the f32r rate, so the serial phase-A head shrinks to ~7us and V/Q
    projections stream inside the attention windows.
  - Scores are fp8 DoubleRow: Q^T/K^T are quantized to fp8 by the projection
    epilogue into a [32p, 2slot, n] layout (d split 32+32), one 256-cycle
    matmul per (head, kt). The 1/sqrt(dk) lands in the exp's scale operand.
  - P@V is restructured: E tiles ([k,q] bf16) are the stationary operand and
    V ([k,64] bf16) moves, so each kt costs 65 cols instead of 512. Output
    C is [q, d] per (head, qsub); the softmax sum Z rides as 1-col matmuls
    into a shared PSUM bank (single start=True per bank, pending-zero
    zero-fill for the other accumulators).
  - Normalization becomes per-partition: reciprocal(Z) + tensor_scalar
    multiply -> bf16, then a PE transpose (identity moving) rebuilds the
    [d, q] cpair layout for the output projection (bf16 weights).
  - The whole C/Z phase for sweep (qc, j) is emitted one j-sweep behind the
    scores/exp/mask of (qc, j+1), so kt-boundary dependencies never stall
    the in-order queues; V projection chunks and the output projection of
    the previous qc fill the leftover PE slack.

  - Head: PE p-state warmup on a memset tile, Exp act-table preload, K/Q
    head-tile epilogues on the idle Act engine. Tail: qc3's output
    projection is staged j0-j2 into SBUF early, then folded back into PSUM
    with an identity matmul + j3 delta, epilogue on the idle Act engine.

PSUM: sps 2x2 banks, C 2x1 (transposes re-mark freed psO-ring banks),
Z 1 (alternating 8-col groups, DVE memset-zeroed), proj/outproj ring 1
= 8 banks.

Measured (CoreSim cost model == harness metric): 305605 ns vs 386447
baseline (-20.9%); HW rel err 1.155e-2 (gate 2e-2).
"""
import numpy as np

import concourse.bass as bass
import concourse.mybir as mybir
import concourse.tile as tile
from concourse import bacc

B, S, D, H = 4, 2048, 1024, 16
DK = D // H          # 64
NCORES = 8
HG = 2               # head groups (tensor-parallel factor per batch)
HPG = H // HG        # 8 heads per core
DH = D // HG         # 512 head dims per core
QCN = 4              # q chunks
QCS = S // QCN       # 512
KT = S // 128        # 16 k tiles
F32 = mybir.dt.float32
F8 = mybir.dt.float8e4
BF16 = mybir.dt.bfloat16
DR = mybir.MatmulPerfMode.DoubleRow


def build_nc():
    nc = bacc.Bacc(None)
    # x inputs: [128, 4 step, 2 slot, S] fp8 (din = step*256+slot*128+p);
    # xq/xk ship only the hi plane (2-product projection), xv hi+lo
    xq8 = [nc.declare_dram_parameter("xq80", [128, 4, 2, S], F8, isOutput=False)]
    xk8 = [nc.declare_dram_parameter("xk80", [128, 4, 2, S], F8, isOutput=False)]
    xv8 = [nc.declare_dram_parameter(f"xv8{t}", [128, 4, 2, S], F8, isOutput=False)
           for t in range(2)]
    # weights: [128, 4 step, 2 slot, 512 outcol] hi/lo fp8, x16, out-permuted
    wq8 = [nc.declare_dram_parameter(f"wq8{t}", [128, 4, 2, DH], F8, isOutput=False)
           for t in range(2)]
    wk8 = [nc.declare_dram_parameter(f"wk8{t}", [128, 4, 2, DH], F8, isOutput=False)
           for t in range(2)]
    wv8 = [nc.declare_dram_parameter(f"wv8{t}", [128, 4, 2, DH], F8, isOutput=False)
           for t in range(2)]
    maskT = nc.declare_dram_parameter("maskT", [S, S], BF16, isOutput=False)
    woT = nc.declare_dram_parameter("woT", [DH, D], BF16, isOutput=False)
    # bqk: cols 0-3 = bq' per (s,half) tile, cols 4-7 = bk'
    bqk = nc.declare_dram_parameter("bqk", [128, 8], F32, isOutput=False)
    vr2 = nc.declare_dram_parameter("vr2", [128, D // 128], F32, isOutput=False)
    ident = nc.declare_dram_parameter("ident", [128, 128], BF16, isOutput=False)
    outT = nc.declare_dram_parameter("outT", [D, S], BF16, isOutput=True)

    with tile.TileContext(nc) as tc:
        with (
            tc.tile_pool(name="persist", bufs=1) as pp,
            tc.tile_pool(name="work", bufs=2) as wp,
            tc.tile_pool(name="psS", bufs=2, space="PSUM") as psS,
            tc.tile_pool(name="psC", bufs=2, space="PSUM") as psC,
            tc.tile_pool(name="psZ", bufs=1, space="PSUM") as psZ,
            tc.tile_pool(name="psO", bufs=1, space="PSUM") as psO,
        ):
            # ---------------- persistent tiles ----------------
            # K^T fp8 per sweep s: [128 (m*32+d32), 2 half, S]
            k8 = [pp.tile([128, 2, S], F8, tag=f"k8_{s}", name=f"k8_{s}")
                  for s in range(2)]
            # V: [128 k, kt, 8h*65] bf16 (64 vals + ones col per head)
            v_full = pp.tile([128, KT, HPG * 65], BF16, tag="v", name="v_full")
            v_sb = [v_full[:, i, :] for i in range(KT)]
            wo_sb = pp.tile([128, 4, D], BF16, tag="wo", name="wo_sb")
            bias_sb = pp.tile([128, 16], F32, tag="bias", name="bias_sb")
            bq_sb = bias_sb[:, 0:4]
            bk_sb = bias_sb[:, 4:8]
            vr_sb = bias_sb[:, 8:16]
            id_sb = pp.tile([128, 128], BF16, tag="ident", name="id_sb")
            wq_sb = [pp.tile([128, 4, 2, DH], F8, tag=f"wq{t}", name=f"wq_sb{t}")
                     for t in range(2)]
            wk_sb = [pp.tile([128, 4, 2, DH], F8, tag=f"wk{t}", name=f"wk_sb{t}")
                     for t in range(2)]
            wv_sb = [pp.tile([128, 4, 2, DH], F8, tag=f"wv{t}", name=f"wv_sb{t}")
                     for t in range(2)]

            # ---------------- boot DMAs ----------------
            # PE p-state warmup on a memset tile: no DMA dependency, so the
            # PE busy period starts immediately and the ramp completes
            # before the first real projection tile
            warm_in = pp.tile([128, 128], BF16, tag="warmin", name="warm_in")
            nc.gpsimd.memset(warm_in[:], 0.25)
            warm_ps = psO.tile([128, QCS], F32, tag="po", name="warm_ps")
            for _ in range(12):
                nc.tensor.matmul(
                    warm_ps[:, 0:128], warm_in[:], warm_in[:],
                    start=True, stop=True, skip_group_check=True)
            # wk planes split across both queues so the first K tile can
            # start as early as possible
            nc.sync.dma_start(wk_sb[0][:], wk8[0][:])
            nc.gpsimd.dma_start(wk_sb[1][:], wk8[1][:])
            nc.gpsimd.dma_start(bias_sb[:, 0:8], bqk[:])
            # ones columns of v_full
            ones_view = v_full.rearrange("p t (h c) -> p t h c", h=HPG)[:, :, :, 64:65]
            nc.gpsimd.memset(ones_view, 1.0)
            # preload the Exp activation table off the critical path
            warm_sb = pp.tile([128, 4], F32, tag="warm", name="warm_sb")
            nc.scalar.activation(
                warm_sb[0:1, 0:1], bias_sb[0:1, 0:1],
                mybir.ActivationFunctionType.Exp)

            # chunk-ring staging for the fp8 x streams
            xk_chunks, xq_chunks, xv_chunks = {}, {}, {}
            XBUFS = {"xk": 3, "xq": 2, "xv": 2}

            def xdma(eng, chunks, src, c, tag):
                pair = tuple(
                    wp.tile([128, 4, 2, QCS], F8, tag=f"{tag}{t}",
                            name=f"{tag}_t{t}", bufs=XBUFS[tag])
                    for t in range(len(src)))
                cols = slice(c * QCS, (c + 1) * QCS)
                for t in range(len(src)):
                    eng.dma_start(pair[t][:], src[t][:, :, :, cols])
                chunks[c] = pair

            def dr_prods(ps, wsb, xpair, prods):
                """DoubleRow matmuls: 4 steps x the given (w,x) plane
                products, accumulating stationary w x moving x into ps."""
                first = True
                n = 0
                total = 4 * len(prods)
                for i in range(4):
                    for (tw, tx) in prods:
                        n += 1
                        nc.tensor.matmul(
                            ps[:], wsb[tw][:, i, :, :],
                            xpair[tx][:, i, :, :],
                            start=first, stop=(n == total), perf_mode=DR)
                        first = False

            # per-(s,half) psum tile from the shared psO ring
            def proj_tile(tag="po"):
                return psO.tile([128, QCS], F32, tag=tag, name="proj_ps")

            QK_PRODS = ((0, 0), (1, 0))   # (w_hi + w_lo) x x_hi
            V_PRODS = ((0, 0), (0, 1), (1, 0))
            _tsp_alt = [0]

            def tsp_eng():
                """alternate projection epilogues between DVE and Pool so
                neither becomes the convoy for dependent scores"""
                return nc.vector

            def emit_kproj_tile(c, s, half, act_epi=False):
                """K projection, one (s,half) out tile of k-chunk c."""
                cols = slice(c * QCS, (c + 1) * QCS)
                t = s * 2 + half
                ps = proj_tile()
                dr_prods(ps,
                         [wk_sb[0][:, :, :, t * 128:(t + 1) * 128],
                          wk_sb[1][:, :, :, t * 128:(t + 1) * 128]],
                         xk_chunks[c], QK_PRODS)
                if act_epi:
                    # head only: Act is still idle there
                    nc.scalar.activation(
                        k8[s][:, half, cols], ps[:],
                        mybir.ActivationFunctionType.Identity,
                        scale=1.0 / 16.0, bias=bk_sb[:, t:t + 1])
                else:
                    nc.vector.tensor_scalar(
                        k8[s][:, half, cols], ps[:], 1.0 / 16.0,
                        bk_sb[:, t:t + 1],
                        mybir.AluOpType.mult, mybir.AluOpType.add)

            q8_tiles = {}
            qproj_done = {}

            def emit_qproj_tile(qc, s, half, act_epi=False):
                qproj_done[qc] = qproj_done.get(qc, 0) + 1
                if qc not in q8_tiles:
                    q8_tiles[qc] = wp.tile([128, 2, 2, QCS], F8, tag="q8",
                                           name="q8_t", bufs=2)
                q8_t = q8_tiles[qc]
                t = s * 2 + half
                ps = proj_tile()
                dr_prods(ps,
                         [wq_sb[0][:, :, :, t * 128:(t + 1) * 128],
                          wq_sb[1][:, :, :, t * 128:(t + 1) * 128]],
                         xq_chunks[qc], QK_PRODS)
                if act_epi:
                    nc.scalar.activation(
                        q8_t[:, s, half, :], ps[:],
                        mybir.ActivationFunctionType.Identity,
                        scale=1.0 / 16.0, bias=bq_sb[:, t:t + 1])
                else:
                    nc.vector.tensor_scalar(
                        q8_t[:, s, half, :], ps[:], 1.0 / 16.0,
                        bq_sb[:, t:t + 1],
                        mybir.AluOpType.mult, mybir.AluOpType.add)

            def emit_vproj_tile(ksub):
                """V projection for one 128-k subtile: out [128, 512 feat]."""
                xv_pair = xv_chunks[ksub // 4]
                kcols = slice((ksub % 4) * 128, (ksub % 4 + 1) * 128)
                ps = proj_tile()
                first = True
                n = 0
                for i in range(4):
                    for (tw, tx) in V_PRODS:
                        n += 1
                        nc.tensor.matmul(
                            ps[:], xv_pair[tx][:, i, :, kcols],
                            wv_sb[tw][:, i, :, :],
                            start=first, stop=(n == 12), perf_mode=DR)
                        first = False
                vdst = v_sb[ksub].rearrange(
                    "p (h c) -> p h c", h=HPG)[:, :, 0:64]
                nc.vector.tensor_scalar(
                    vdst, ps[:].rearrange("p (h c) -> p h c", h=HPG),
                    1.0 / 16.0, 0.0,
                    mybir.AluOpType.mult, mybir.AluOpType.add)

            def load_mask(qc):
                mask_sb = wp.tile([128, KT, QCS], BF16, tag="mask",
                                  name="mask_sb", bufs=2)
                ms = maskT[:, qc * QCS:(qc + 1) * QCS].rearrange(
                    "(t p) s -> p t s", p=128)
                hm = KT // 2
                nc.sync.dma_start(mask_sb[:, 0:hm, :], ms[:, 0:hm, :])
                nc.gpsimd.dma_start(mask_sb[:, hm:KT, :], ms[:, hm:KT, :])
                return mask_sb

            # ---------------- phase A: minimal head ----------------
            xdma(nc.sync, xk_chunks, xk8, 0, "xk")
            xdma(nc.sync, xq_chunks, xq8, 0, "xq")
            for t in range(2):
                nc.gpsimd.dma_start(wq_sb[t][:], wq8[t][:])
            nc.gpsimd.dma_start(id_sb[:], ident[:])
            nc.gpsimd.dma_start(vr_sb[:, :], vr2[:])
            # head: only the sweep-0 tiles attention j0 needs immediately;
            # their epilogues ride the still-idle Act engine
            emit_kproj_tile(0, 0, 0, act_epi=True)
            emit_kproj_tile(0, 0, 1, act_epi=True)
            emit_qproj_tile(0, 0, 0, act_epi=True)
            emit_qproj_tile(0, 0, 1, act_epi=True)
            mask0 = load_mask(0)
            xdma(nc.sync, xk_chunks, xk8, 1, "xk")
            xdma(nc.sync, xk_chunks, xk8, 2, "xk")
            for t in range(2):
                nc.gpsimd.dma_start(wv_sb[t][:], wv8[t][:])
            xdma(nc.gpsimd, xv_chunks, xv8, 0, "xv")
            xdma(nc.sync, xv_chunks, xv8, 1, "xv")
            nc.gpsimd.dma_start(
                wo_sb[:],
                woT.rearrange("(j p) d -> p j d", p=128))

            # ---------------- phase B ----------------
            # sweep state carried between windows
            sweeps = {}   # (qc, j) -> dict(e=..list of e tiles.., ..)
            cpairs = {}   # (qc, j) -> cpair AP

            def emit_scores_exp_mask(qc, j, kt, mask_sb, q8_t):
                sps = psS.tile([128, 2, QCS], F32, tag="sps", name="sps", bufs=2)
                for hh in range(2):
                    h = 2 * j + hh
                    s, m = h // 4, h % 4
                    nc.tensor.matmul(
                        sps[:, hh, :],
                        k8[s][32 * m:32 * m + 32, :, kt * 128:(kt + 1) * 128],
                        q8_t[32 * m:32 * m + 32, s, :, :],
                        start=True, stop=True, perf_mode=DR,
                        tile_position=(32 * m, 0))
                e_sb = wp.tile([128, 2, QCS], BF16, tag="e", name="e_sb", bufs=30)
                nc.scalar.activation(
                    e_sb[:], sps[:], mybir.ActivationFunctionType.Exp,
                    scale=0.125)
                meng = nc.vector if kt % 2 == 0 else nc.gpsimd
                for hh in range(2):
                    meng.tensor_mul(e_sb[:, hh, :], e_sb[:, hh, :],
                                    mask_sb[:, kt, :])
                return e_sb

            # single persistent Z bank; sweeps alternate 8-col groups
            z_bank = psZ.tile([128, QCS], F32, tag="z", name="z_bank")

            def start_sweep(qc, j):
                n = 4 * qc + j
                cps = psC.tile([128, QCS], F32, tag="c", name="cps", bufs=2)
                zoff = 256 + 8 * (n % 2)
                return {"qc": qc, "j": j, "c": cps, "z": z_bank, "zoff": zoff,
                        "n": 0}

            def emit_cz(sw, e_sb, kt):
                """C and Z matmuls for one kt of the lagged sweep."""
                j, cps = sw["j"], sw["c"]
                zb, zoff = sw["z"], sw["zoff"]
                for hh in range(2):
                    h = 2 * j + hh
                    for qsub in range(4):
                        idx = hh * 4 + qsub
                        est = e_sb[:, hh, qsub * 128:(qsub + 1) * 128]
                        first = (sw["n"] == 0)
                        sw["n"] += 1
                        nc.tensor.matmul(
                            cps[:, idx * 64:(idx + 1) * 64], est,
                            v_sb[kt][:, h * 65:h * 65 + 64],
                            start=first, stop=(kt == KT - 1 and idx == 7),
                            skip_group_check=True)
                        nc.tensor.matmul(
                            zb[:, zoff + idx:zoff + idx + 1], est,
                            v_sb[kt][:, h * 65 + 64:h * 65 + 65],
                            start=False, stop=(kt == KT - 1 and idx == 7),
                            skip_group_check=True)

            def emit_sweep_epilogue(sw):
                """recip Z, normalize -> bf16, PE transpose (into the C bank
                after its stripes are consumed), cpair copy."""
                qc, j, cps = sw["qc"], sw["j"], sw["c"]
                zb, zoff = sw["z"], sw["zoff"]
                rz = wp.tile([128, 8], F32, tag="rz", name="rz", bufs=2)
                nc.vector.reciprocal_approx_fast(
                    out=rz[:], in_=zb[:, zoff:zoff + 8])
                cn = wp.tile([128, 8, 64], BF16, tag="cn", name="cn", bufs=2)
                for idx in range(8):
                    nc.vector.tensor_scalar(
                        cn[:, idx, :], cps[:, idx * 64:(idx + 1) * 64],
                        rz[:, idx:idx + 1], 0.0,
                        mybir.AluOpType.mult, mybir.AluOpType.add)
                # transposes stage through a transient psO-ring tile: the
                # first one's start=True zeroes the bank, the rest rely on
                # the pending-zero it leaves (same pattern as Z/C banks)
                ct = psO.tile([128, QCS], BF16, tag="po", name="ct")
                first = True
                for hh in range(2):
                    for qsub in range(4):
                        idx = hh * 4 + qsub
                        nc.tensor.matmul(
                            ct[hh * 64:hh * 64 + 64,
                               qsub * 128:(qsub + 1) * 128],
                            cn[:, idx, :], id_sb[:],
                            is_transpose=True, start=first, stop=True,
                            skip_group_check=True)
                        first = False
                cpair = wp.tile([128, QCS], BF16, tag="cpair", name="cpair",
                                bufs=5)
                nc.vector.tensor_copy(cpair[:], ct[:])
                cpairs[(qc, j)] = cpair

            def emit_outproj_ot(qc, ot):
                cp = [cpairs[(qc, j)] for j in range(4)]
                po = psO.tile([128, QCS], F32, tag="po", name="po")
                for j in range(4):
                    nc.tensor.matmul(
                        po[:], wo_sb[:, j, ot * 128:(ot + 1) * 128],
                        cp[j][:], start=(j == 0), stop=(j == 3))
                o_sb = wp.tile([128, QCS], BF16, tag="o", name="o_sb", bufs=3)
                nc.vector.tensor_scalar(
                    o_sb[:], po[:], 1.0, vr_sb[:, ot:ot + 1],
                    mybir.AluOpType.mult, mybir.AluOpType.add)
                eng = nc.gpsimd if ot % 2 else nc.sync
                eng.dma_start(
                    outT[ot * 128:(ot + 1) * 128, qc * QCS:(qc + 1) * QCS],
                    o_sb[:])

            # ---- budgeted PE work queue ----
            # Items: (cost_ns, thunk). Emitted in FIFO order, paced so each
            # slot adds at most ~BUDGET ns of PE work on top of the scores.
            from collections import deque
            work = deque()
            BUDGET = 800.0
            CAP = 2000.0
            allowance = [0.0]

            def kt_(f, *a, **kw):
                return lambda: f(*a, **kw)

            def push(cost, thunk):
                work.append((cost, thunk))

            def drain_slot():
                allowance[0] = min(allowance[0] + BUDGET, CAP)
                while work and allowance[0] >= work[0][0]:
                    cost, thunk = work.popleft()
                    allowance[0] -= cost
                    thunk()

            def drain_all():
                while work:
                    _, thunk = work.popleft()
                    thunk()

            # fixed j0 slot tasks: the K tiles attention depends on (hard
            # deadlines), chunk-paced with their ring DMAs. Fixed slot k
            # fires right after scores(k+1) is emitted; the sweep-0 half of
            # chunk c must land before scores(kt=4c).
            fixed = {}
            fixed[(0, 0, 0)] = [kt_(emit_kproj_tile, 1, 0, 0)]
            fixed[(0, 0, 1)] = [kt_(emit_kproj_tile, 1, 0, 1)]
            fixed[(0, 0, 2)] = [kt_(emit_kproj_tile, 0, 1, 0)]
            fixed[(0, 0, 3)] = [kt_(emit_kproj_tile, 0, 1, 1)]
            fixed[(0, 0, 4)] = [kt_(xdma, nc.sync, xk_chunks, xk8, 3, "xk"),
                                kt_(emit_kproj_tile, 2, 0, 0)]
            fixed[(0, 0, 5)] = [kt_(emit_kproj_tile, 2, 0, 1)]
            fixed[(0, 0, 6)] = [kt_(emit_kproj_tile, 1, 1, 0)]
            fixed[(0, 0, 7)] = [kt_(emit_kproj_tile, 1, 1, 1)]
            fixed[(0, 0, 8)] = [kt_(emit_kproj_tile, 3, 0, 0)]
            fixed[(0, 0, 9)] = [kt_(emit_kproj_tile, 3, 0, 1)]
            fixed[(0, 0, 10)] = [kt_(emit_kproj_tile, 2, 1, 0)]
            fixed[(0, 0, 11)] = [kt_(emit_kproj_tile, 2, 1, 1)]
            fixed[(0, 0, 12)] = [kt_(emit_kproj_tile, 3, 1, 0)]
            fixed[(0, 0, 13)] = [kt_(emit_kproj_tile, 3, 1, 1)]
            fixed[(0, 0, 14)] = [kt_(emit_qproj_tile, 0, 1, 0)]
            fixed[(0, 0, 15)] = [kt_(emit_qproj_tile, 0, 1, 1)]
            for qcn in range(1, QCN):
                eng_x = nc.gpsimd if qcn % 2 else nc.sync
                fixed[(qcn - 1, 1, 12)] = [
                    kt_(xdma, eng_x, xq_chunks, xq8, qcn, "xq")]

            TILE_NS = 1300.0
            CZ_NS = 230.0
            EPI_NS = 900.0
            OT_NS = 880.0

            epi_count = [0]
            oa_tiles = {}

            def emit_outproj_partial(ot):
                """qc3 ots, j0-j2 partial accumulated early -> bf16 SBUF"""
                cp = [cpairs[(QCN - 1, j)] for j in range(3)]
                po = psO.tile([128, QCS], F32, tag="po", name="po")
                for j in range(3):
                    nc.tensor.matmul(
                        po[:], wo_sb[:, j, ot * 128:(ot + 1) * 128],
                        cp[j][:], start=(j == 0), stop=(j == 2))
                oa = wp.tile([128, QCS], BF16, tag="oa", name="oa", bufs=8)
                nc.vector.tensor_scalar(
                    oa[:], po[:], 1.0, 0.0,
                    mybir.AluOpType.mult, mybir.AluOpType.add)
                oa_tiles[ot] = oa

            def emit_epi(sw):
                emit_sweep_epilogue(sw)
                epi_count[0] += 1
                qc, j = sw["qc"], sw["j"]
                if j == 3 and qc < QCN - 1:
                    for ot in range(8):
                        push(OT_NS, kt_(emit_outproj_ot, qc, ot))
                if qc == QCN - 1 and j == 2:
                    for ot in range(8):
                        push(OT_NS, kt_(emit_outproj_partial, ot))

            def push_sweep(sw):
                """queue the CZ phase + epilogue of a finished sweep
                (early sweeps only; later sweeps inline their CZ)."""
                qc, j = sw["qc"], sw["j"]
                zb, zoff = sw["z"], sw["zoff"]
                # zero this sweep's Z col-group (queued, so it lands after
                # the PREVIOUS same-group sweep's Z reads)
                push(0, kt_(nc.vector.memset, zb[:, zoff:zoff + 8], 0.0))
                if qc == 0 and j == 0:
                    # V tiles interleave just ahead of the CZ kts that
                    # consume them (V projection happens here, in the lag
                    # window, not in phase A)
                    for kt in range(KT):
                        push(TILE_NS, kt_(emit_vproj_tile, kt))
                        push(CZ_NS, kt_(emit_cz, sw, sw["e"][kt], kt))
                        if kt == 3:
                            push(0, kt_(xdma, nc.gpsimd, xv_chunks, xv8,
                                        2, "xv"))
                        if kt == 7:
                            push(0, kt_(xdma, nc.sync, xv_chunks, xv8,
                                        3, "xv"))
                else:
                    for kt in range(KT):
                        push(CZ_NS, kt_(emit_cz, sw, sw["e"][kt], kt))
                push(EPI_NS, kt_(emit_epi, sw))

            INLINE_FROM = 4   # sweeps with index >= this inline their CZ
            LAG = 3
            mask_next = mask0
            for qc in range(QCN):
                mask_sb = mask_next
                if qc > 0:
                    # this qc's Q tiles must be emitted before its scores
                    while qproj_done.get(qc, 0) < 4:
                        _, thunk = work.popleft()
                        thunk()
                for j in range(4):
                    n = 4 * qc + j
                    q8_t = q8_tiles[qc]
                    sw = start_sweep(qc, j)
                    inline = (n >= INLINE_FROM)
                    if inline:
                        # predecessors of this sweep's C bank and Z group
                        # must be fully consumed before we touch them
                        while epi_count[0] < n - 1:
                            _, thunk = work.popleft()
                            thunk()
                        nc.vector.memset(
                            z_bank[:, sw["zoff"]:sw["zoff"] + 8], 0.0)
                    for kt in range(KT):
                        e_sb = emit_scores_exp_mask(qc, j, kt, mask_sb, q8_t)
                        sw.setdefault("e", []).append(e_sb)
                        # lookahead: scores/exp/mask of kt are emitted before
                        # the budgeted work of kt-1's slot
                        if kt > 0:
                            for thunk in fixed.pop((qc, j, kt - 1), ()):
                                thunk()
                            drain_slot()
                        if inline and kt >= LAG:
                            emit_cz(sw, sw["e"][kt - LAG], kt - LAG)
                        if j == 3 and kt == 3 and qc + 1 < QCN:
                            mask_next = load_mask(qc + 1)
                        if j == 2 and kt in (2, 5, 8, 11) and qc + 1 < QCN:
                            push(TILE_NS, kt_(emit_qproj_tile, qc + 1,
                                              (kt - 2) // 6, ((kt - 2) // 3) % 2))

                    for thunk in fixed.pop((qc, j, KT - 1), ()):
                        thunk()
                    drain_slot()
                    if inline:
                        for kt in range(KT - LAG, KT):
                            emit_cz(sw, sw["e"][kt], kt)
                        emit_epi(sw)
                    else:
                        push_sweep(sw)
                q8_tiles.pop(qc)
            drain_all()
            # final output projection (qc3): j3 delta + identity-matmul
            # fold of the staged j0-j2 partial, epilogue on the now-idle
            # Act engine, pipelined through both free PSUM rings
            for ot in range(8):
                if ot % 2:
                    po = psO.tile([128, QCS], F32, tag="po", name="po2")
                else:
                    po2 = psS.tile([128, 2, QCS], F32, tag="sps", name="po2",
                                   bufs=2)
                    po = po2[:, 0, :]
                nc.tensor.matmul(
                    po[:], wo_sb[:, 3, ot * 128:(ot + 1) * 128],
                    cpairs[(QCN - 1, 3)][:], start=True, stop=False)
                nc.tensor.matmul(
                    po[:], id_sb[:], oa_tiles[ot][:],
                    start=False, stop=True)
                o_sb = wp.tile([128, QCS], BF16, tag="o", name="o_sb", bufs=3)
                nc.scalar.activation(
                    o_sb[:], po[:], mybir.ActivationFunctionType.Identity,
                    bias=vr_sb[:, ot:ot + 1])
                eng = nc.gpsimd if ot % 2 else nc.sync
                eng.dma_start(
                    outT[ot * 128:(ot + 1) * 128,
                         (QCN - 1) * QCS:QCN * QCS],
                    o_sb[:])

    nc.finalize()
    return nc


_NC_CACHE = None


def _get_nc():
    global _NC_CACHE
    if _NC_CACHE is None:
        _NC_CACHE = build_nc()
    return _NC_CACHE


def _hi_lo_fp8(x):
    import ml_dtypes
    f8 = ml_dtypes.float8_e4m3
    hi = x.astype(f8)
    lo = (x - hi.astype(np.float32)).astype(f8)
    return hi, lo


def _x_prep(xT):
    """[D, S] f32 -> ([128, 4, 2, S] hi, lo) fp8 with din=i*256+s*128+p."""
    r = xT.reshape(4, 2, 128, xT.shape[1]).transpose(2, 0, 1, 3)
    return _hi_lo_fp8(np.ascontiguousarray(r))


def _w_prep(w_slice, perm):
    """w_slice [512 outf, 1024 din] -> ([128, 4, 2, 512] hi, lo) fp8 x16.
    Column c of the output = out-feature perm[c]."""
    w = (16.0 * w_slice[perm]).T          # [1024 din, 512 outcol]
    r = w.reshape(4, 2, 128, 512).transpose(2, 0, 1, 3)
    return _hi_lo_fp8(np.ascontiguousarray(r))


def shard_inputs(query, key, value, mask, wq, bq, wk, bk, wv, bv, wo, bo):
    import ml_dtypes
    bf = ml_dtypes.bfloat16

    query = np.asarray(query, np.float32)
    key = np.asarray(key, np.float32)
    value = np.asarray(value, np.float32)
    mask = np.asarray(mask)
    wq = np.asarray(wq, np.float32); bq = np.asarray(bq, np.float32)
    wk = np.asarray(wk, np.float32); bk = np.asarray(bk, np.float32)
    wv = np.asarray(wv, np.float32); bv = np.asarray(bv, np.float32)
    wo = np.asarray(wo, np.float32); bo = np.asarray(bo, np.float32)

    # out-feature permutation for Q/K: col (t, oc) -> f = (4s+oc//32)*64
    #  + 32*half + oc%32, t = 2s+half
    perm = np.empty(512, np.int64)
    for s_ in range(2):
        for half in range(2):
            t = 2 * s_ + half
            oc = np.arange(128)
            perm[t * 128:(t + 1) * 128] = ((4 * s_ + oc // 32) * 64
                                           + 32 * half + oc % 32)

    ident = np.eye(128, dtype=np.float32).astype(bf)
    maskT_b = [np.ascontiguousarray(mask[b].T).astype(bf) for b in range(B)]
    xp = {}
    for b in range(B):
        xp[b] = (
            _x_prep(np.ascontiguousarray(query[b].T)),
            _x_prep(np.ascontiguousarray(key[b].T)),
            _x_prep(np.ascontiguousarray(value[b].T)),
        )

    in_maps = []
    for c in range(NCORES):
        b, hg = divmod(c, HG)
        sl = slice(hg * DH, (hg + 1) * DH)
        wo_block = wo[:, sl]                       # [1024, 512]
        v_r = bv[sl] @ wo_block.T                  # [1024]
        if hg == 0:
            v_r = v_r + bo
        wq_hl = _w_prep(wq[sl], perm)
        wk_hl = _w_prep(wk[sl], perm)
        wv_hl = _w_prep(wv[sl], np.arange(512))
        bqk_arr = np.zeros((128, 8), np.float32)
        for t in range(4):
            p = np.arange(128)
            f = perm[t * 128 + p]
            bqk_arr[:, t] = bq[sl][f]
            bqk_arr[:, 4 + t] = bk[sl][f]
        (xq_hi, _), (xk_hi, _), (xv_hi, xv_lo) = xp[b]
        in_maps.append({
            "xq80": xq_hi,
            "xk80": xk_hi,
            "xv80": xv_hi, "xv81": xv_lo,
            "wq80": wq_hl[0], "wq81": wq_hl[1],
            "wk80": wk_hl[0], "wk81": wk_hl[1],
            "wv80": wv_hl[0], "wv81": wv_hl[1],
            "maskT": maskT_b[b],
            "woT": np.ascontiguousarray(wo_block.T).astype(bf),
            "bqk": bqk_arr,
            "vr2": np.ascontiguousarray(v_r.reshape(D // 128, 128).T),
            "ident": ident,
        })
    return in_maps


def combine_outputs(results):
    """results: list of per-core {"outT": [1024, 2048] bf16} -> [B, S, D]."""
    out = np.empty((B, S, D), np.float32)
    for b in range(B):
        acc = (results[2 * b]["outT"].astype(np.float32)
               + results[2 * b + 1]["outT"].astype(np.float32))
        out[b] = acc.T
    return out


def kernel(**inputs):
    from concourse.bass_utils import run_bass_kernel_spmd

    nc = _get_nc()
    in_maps = shard_inputs(**inputs)
    res = run_bass_kernel_spmd(nc, in_maps, list(range(NCORES)))
    return combine_outputs(res.results)


# revision 75
# speedup vs baseline: 1.2654x; 1.0007x over previous
"""Multi-head attention Trainium2 Bass kernel, sharded over 8 NeuronCores.

Problem: B=4, S=2048, D=1024, H=16 heads (DK=64), fp32, random 0/1 mask.

Sharding (data-parallel batch x tensor-parallel heads):
  core c handles batch b = c // 2, head-group hg = c % 2 (8 heads = 512 dims).
  Host sums the two head-group partials per batch.

Design (v2, fp8 DoubleRow):
  The attention inner loop is Activation-engine bound (exp of every score,
  256 x [128,1024] ACTIVATE = ~266us floor). Everything else is scheduled to
  hide under it:
  - Q/K/V projections run as fp8e4 DoubleRow matmuls (hi/lo split of both
    inputs and weights, 3 products, weights pre-scaled x16 on host) at 4x
    the f32r rate, so the serial phase-A head shrinks to ~7us and V/Q
    projections stream inside the attention windows.
  - Scores are fp8 DoubleRow: Q^T/K^T are quantized to fp8 by the projection
    epilogue into a [32p, 2slot, n] layout (d split 32+32), one 256-cycle
    matmul per (head, kt). The 1/sqrt(dk) lands in the exp's scale operand.
  - P@V is restructured: E tiles ([k,q] bf16) are the stationary operand and
    V ([k,64] bf16) moves, so each kt costs 65 cols instead of 512. Output
    C is [q, d] per (head, qsub); the softmax sum Z rides as 1-col matmuls
    into a shared PSUM bank (single start=True per bank, pending-zero
    zero-fill for the other accumulators).
  - Normalization becomes per-partition: reciprocal(Z) + tensor_scalar
    multiply -> bf16, then a PE transpose (identity moving) rebuilds the
    [d, q] cpair layout for the output projection (bf16 weights).
  - The whole C/Z phase for sweep (qc, j) is emitted one j-sweep behind the
    scores/exp/mask of (qc, j+1), so kt-boundary dependencies never stall
    the in-order queues; V projection chunks and the output projection of
    the previous qc fill the leftover PE slack.

  - Head: PE p-state warmup on a memset tile, Exp act-table preload, K/Q
    head-tile epilogues on the idle Act engine. Tail: qc3's output
    projection is staged j0-j2 into SBUF early, then folded back into PSUM
    with an identity matmul + j3 delta, epilogue on the idle Act engine.

PSUM: sps 2x2 banks, C 2x1 (transposes re-mark freed psO-ring banks),
Z 1 (alternating 8-col groups, DVE memset-zeroed), proj/outproj ring 1
= 8 banks.

Measured (CoreSim cost model == harness metric): 305605 ns vs 386447
baseline (-20.9%); HW rel err 1.155e-2 (gate 2e-2).
"""
import numpy as np

import concourse.bass as bass
import concourse.mybir as mybir
import concourse.tile as tile
from concourse import bacc

B, S, D, H = 4, 2048, 1024, 16
DK = D // H          # 64
NCORES = 8
HG = 2               # head groups (tensor-parallel factor per batch)
HPG = H // HG        # 8 heads per core
DH = D // HG         # 512 head dims per core
QCN = 4              # q chunks
QCS = S // QCN       # 512
KT = S // 128        # 16 k tiles
F32 = mybir.dt.float32
F8 = mybir.dt.float8e4
BF16 = mybir.dt.bfloat16
DR = mybir.MatmulPerfMode.DoubleRow


def build_nc():
    nc = bacc.Bacc(None)
    # x inputs: [128, 4 step, 2 slot, S] fp8 (din = step*256+slot*128+p);
    # xq/xk ship only the hi plane (2-product projection), xv hi+lo
    xq8 = [nc.declare_dram_parameter("xq80", [128, 4, 2, S], F8, isOutput=False)]
    xk8 = [nc.declare_dram_parameter("xk80", [128, 4, 2, S], F8, isOutput=False)]
    xv8 = [nc.declare_dram_parameter(f"xv8{t}", [128, 4, 2, S], F8, isOutput=False)
           for t in range(2)]
    # weights: [128, 4 step, 2 slot, 512 outcol] hi/lo fp8, x16, out-permuted
    wq8 = [nc.declare_dram_parameter(f"wq8{t}", [128, 4, 2, DH], F8, isOutput=False)
           for t in range(2)]
    wk8 = [nc.declare_dram_parameter(f"wk8{t}", [128, 4, 2, DH], F8, isOutput=False)
           for t in range(2)]
    wv8 = [nc.declare_dram_parameter(f"wv8{t}", [128, 4, 2, DH], F8, isOutput=False)
           for t in range(2)]
    maskT = nc.declare_dram_parameter("maskT", [S, S], BF16, isOutput=False)
    woT = nc.declare_dram_parameter("woT", [DH, D], BF16, isOutput=False)
    # bqk: cols 0-3 = bq' per (s,half) tile, cols 4-7 = bk'
    bqk = nc.declare_dram_parameter("bqk", [128, 8], F32, isOutput=False)
    vr2 = nc.declare_dram_parameter("vr2", [128, D // 128], F32, isOutput=False)
    ident = nc.declare_dram_parameter("ident", [128, 128], BF16, isOutput=False)
    outT = nc.declare_dram_parameter("outT", [D, S], BF16, isOutput=True)

    with tile.TileContext(nc) as tc:
        with (
            tc.tile_pool(name="persist", bufs=1) as pp,
            tc.tile_pool(name="work", bufs=2) as wp,
            tc.tile_pool(name="psS", bufs=2, space="PSUM") as psS,
            tc.tile_pool(name="psC", bufs=2, space="PSUM") as psC,
            tc.tile_pool(name="psZ", bufs=1, space="PSUM") as psZ,
            tc.tile_pool(name="psO", bufs=1, space="PSUM") as psO,
        ):
            # ---------------- persistent tiles ----------------
            # K^T fp8 per sweep s: [128 (m*32+d32), 2 half, S]
            k8 = [pp.tile([128, 2, S], F8, tag=f"k8_{s}", name=f"k8_{s}")
                  for s in range(2)]
            # V: [128 k, kt, 8h*65] bf16 (64 vals + ones col per head)
            v_full = pp.tile([128, KT, HPG * 65], BF16, tag="v", name="v_full")
            v_sb = [v_full[:, i, :] for i in range(KT)]
            wo_sb = pp.tile([128, 4, D], BF16, tag="wo", name="wo_sb")
            bias_sb = pp.tile([128, 16], F32, tag="bias", name="bias_sb")
            bq_sb = bias_sb[:, 0:4]
            bk_sb = bias_sb[:, 4:8]
            vr_sb = bias_sb[:, 8:16]
            id_sb = pp.tile([128, 128], BF16, tag="ident", name="id_sb")
            wq_sb = [pp.tile([128, 4, 2, DH], F8, tag=f"wq{t}", name=f"wq_sb{t}")
                     for t in range(2)]
            wk_sb = [pp.tile([128, 4, 2, DH], F8, tag=f"wk{t}", name=f"wk_sb{t}")
                     for t in range(2)]
            wv_sb = [pp.tile([128, 4, 2, DH], F8, tag=f"wv{t}", name=f"wv_sb{t}")
                     for t in range(2)]

            # ---------------- boot DMAs ----------------
            # PE p-state warmup on a memset tile: no DMA dependency, so the
            # PE busy period starts immediately and the ramp completes
            # before the first real projection tile
            warm_in = pp.tile([128, 128], BF16, tag="warmin", name="warm_in")
            nc.gpsimd.memset(warm_in[:], 0.25)
            warm_ps = psO.tile([128, QCS], F32, tag="po", name="warm_ps")
            for _ in range(30):
                nc.tensor.matmul(
                    warm_ps[:, 0:128], warm_in[:], warm_in[:],
                    start=True, stop=True, skip_group_check=True)
            # wk planes split across both queues so the first K tile can
            # start as early as possible
            nc.sync.dma_start(wk_sb[0][:], wk8[0][:])
            nc.gpsimd.dma_start(wk_sb[1][:], wk8[1][:])
            nc.gpsimd.dma_start(bias_sb[:, 0:8], bqk[:])
            # ones columns of v_full
            ones_view = v_full.rearrange("p t (h c) -> p t h c", h=HPG)[:, :, :, 64:65]
            nc.gpsimd.memset(ones_view, 1.0)
            # preload the Exp activation table off the critical path
            warm_sb = pp.tile([128, 4], F32, tag="warm", name="warm_sb")
            nc.scalar.activation(
                warm_sb[0:1, 0:1], bias_sb[0:1, 0:1],
                mybir.ActivationFunctionType.Exp)

            # chunk-ring staging for the fp8 x streams
            xk_chunks, xq_chunks, xv_chunks = {}, {}, {}
            XBUFS = {"xk": 3, "xq": 2, "xv": 2}

            def xdma(eng, chunks, src, c, tag):
                pair = tuple(
                    wp.tile([128, 4, 2, QCS], F8, tag=f"{tag}{t}",
                            name=f"{tag}_t{t}", bufs=XBUFS[tag])
                    for t in range(len(src)))
                cols = slice(c * QCS, (c + 1) * QCS)
                for t in range(len(src)):
                    eng.dma_start(pair[t][:], src[t][:, :, :, cols])
                chunks[c] = pair

            def dr_prods(ps, wsb, xpair, prods):
                """DoubleRow matmuls: 4 steps x the given (w,x) plane
                products, accumulating stationary w x moving x into ps."""
                first = True
                n = 0
                total = 4 * len(prods)
                for i in range(4):
                    for (tw, tx) in prods:
                        n += 1
                        nc.tensor.matmul(
                            ps[:], wsb[tw][:, i, :, :],
                            xpair[tx][:, i, :, :],
                            start=first, stop=(n == total), perf_mode=DR)
                        first = False

            # per-(s,half) psum tile from the shared psO ring
            def proj_tile(tag="po"):
                return psO.tile([128, QCS], F32, tag=tag, name="proj_ps")

            QK_PRODS = ((0, 0), (1, 0))   # (w_hi + w_lo) x x_hi
            V_PRODS = ((0, 0), (0, 1), (1, 0))
            _tsp_alt = [0]

            def tsp_eng():
                """alternate projection epilogues between DVE and Pool so
                neither becomes the convoy for dependent scores"""
                return nc.vector

            def emit_kproj_tile(c, s, half, act_epi=False):
                """K projection, one (s,half) out tile of k-chunk c."""
                cols = slice(c * QCS, (c + 1) * QCS)
                t = s * 2 + half
                ps = proj_tile()
                dr_prods(ps,
                         [wk_sb[0][:, :, :, t * 128:(t + 1) * 128],
                          wk_sb[1][:, :, :, t * 128:(t + 1) * 128]],
                         xk_chunks[c], QK_PRODS)
                if act_epi:
                    # head only: Act is still idle there
                    nc.scalar.activation(
                        k8[s][:, half, cols], ps[:],
                        mybir.ActivationFunctionType.Identity,
                        scale=1.0 / 16.0, bias=bk_sb[:, t:t + 1])
                else:
                    nc.vector.tensor_scalar(
                        k8[s][:, half, cols], ps[:], 1.0 / 16.0,
                        bk_sb[:, t:t + 1],
                        mybir.AluOpType.mult, mybir.AluOpType.add)

            q8_tiles = {}
            qproj_done = {}

            def emit_qproj_tile(qc, s, half, act_epi=False):
                qproj_done[qc] = qproj_done.get(qc, 0) + 1
                if qc not in q8_tiles:
                    q8_tiles[qc] = wp.tile([128, 2, 2, QCS], F8, tag="q8",
                                           name="q8_t", bufs=2)
                q8_t = q8_tiles[qc]
                t = s * 2 + half
                ps = proj_tile()
                dr_prods(ps,
                         [wq_sb[0][:, :, :, t * 128:(t + 1) * 128],
                          wq_sb[1][:, :, :, t * 128:(t + 1) * 128]],
                         xq_chunks[qc], QK_PRODS)
                if act_epi:
                    nc.scalar.activation(
                        q8_t[:, s, half, :], ps[:],
                        mybir.ActivationFunctionType.Identity,
                        scale=1.0 / 16.0, bias=bq_sb[:, t:t + 1])
                else:
                    nc.vector.tensor_scalar(
                        q8_t[:, s, half, :], ps[:], 1.0 / 16.0,
                        bq_sb[:, t:t + 1],
                        mybir.AluOpType.mult, mybir.AluOpType.add)

            _vpart = {}

            def emit_vproj_part(ksub, part):
                """half of a V-projection subtile (6 of 12 DR products), so
                the queue can pace V work in sub-slot chunks"""
                xv_pair = xv_chunks[ksub // 4]
                kcols = slice((ksub % 4) * 128, (ksub % 4 + 1) * 128)
                if part == 0:
                    _vpart[ksub] = proj_tile()
                ps = _vpart[ksub]
                n = 6 * part
                for i in (0, 1) if part == 0 else (2, 3):
                    for (tw, tx) in V_PRODS:
                        n += 1
                        nc.tensor.matmul(
                            ps[:], xv_pair[tx][:, i, :, kcols],
                            wv_sb[tw][:, i, :, :],
                            start=(n == 1), stop=(n == 12), perf_mode=DR)
                if part == 0:
                    return
                del _vpart[ksub]
                vdst = v_sb[ksub].rearrange(
                    "p (h c) -> p h c", h=HPG)[:, :, 0:64]
                nc.vector.tensor_scalar(
                    vdst, ps[:].rearrange("p (h c) -> p h c", h=HPG),
                    1.0 / 16.0, 0.0,
                    mybir.AluOpType.mult, mybir.AluOpType.add)

            def load_mask(qc):
                mask_sb = wp.tile([128, KT, QCS], BF16, tag="mask",
                                  name="mask_sb", bufs=2)
                ms = maskT[:, qc * QCS:(qc + 1) * QCS].rearrange(
                    "(t p) s -> p t s", p=128)
                hm = KT // 2
                nc.sync.dma_start(mask_sb[:, 0:hm, :], ms[:, 0:hm, :])
                nc.gpsimd.dma_start(mask_sb[:, hm:KT, :], ms[:, hm:KT, :])
                return mask_sb

            # ---------------- phase A: minimal head ----------------
            xdma(nc.sync, xk_chunks, xk8, 0, "xk")
            xdma(nc.sync, xq_chunks, xq8, 0, "xq")
            for t in range(2):
                nc.gpsimd.dma_start(wq_sb[t][:], wq8[t][:])
            nc.gpsimd.dma_start(id_sb[:], ident[:])
            nc.gpsimd.dma_start(vr_sb[:, :], vr2[:])
            # head: only the sweep-0 tiles attention j0 needs immediately;
            # their epilogues ride the still-idle Act engine
            emit_kproj_tile(0, 0, 0, act_epi=True)
            emit_kproj_tile(0, 0, 1, act_epi=True)
            emit_qproj_tile(0, 0, 0, act_epi=True)
            emit_qproj_tile(0, 0, 1, act_epi=True)
            mask0 = load_mask(0)
            xdma(nc.sync, xk_chunks, xk8, 1, "xk")
            xdma(nc.sync, xk_chunks, xk8, 2, "xk")
            for t in range(2):
                nc.gpsimd.dma_start(wv_sb[t][:], wv8[t][:])
            xdma(nc.gpsimd, xv_chunks, xv8, 0, "xv")
            xdma(nc.sync, xv_chunks, xv8, 1, "xv")
            nc.gpsimd.dma_start(
                wo_sb[:],
                woT.rearrange("(j p) d -> p j d", p=128))

            # ---------------- phase B ----------------
            # sweep state carried between windows
            sweeps = {}   # (qc, j) -> dict(e=..list of e tiles.., ..)
            cpairs = {}   # (qc, j) -> cpair AP

            def emit_scores_exp_mask(qc, j, kt, mask_sb, q8_t):
                sps = psS.tile([128, 2, QCS], F32, tag="sps", name="sps", bufs=2)
                for hh in range(2):
                    h = 2 * j + hh
                    s, m = h // 4, h % 4
                    nc.tensor.matmul(
                        sps[:, hh, :],
                        k8[s][32 * m:32 * m + 32, :, kt * 128:(kt + 1) * 128],
                        q8_t[32 * m:32 * m + 32, s, :, :],
                        start=True, stop=True, perf_mode=DR,
                        tile_position=(32 * m, 0))
                e_sb = wp.tile([128, 2, QCS], BF16, tag="e", name="e_sb", bufs=30)
                nc.scalar.activation(
                    e_sb[:], sps[:], mybir.ActivationFunctionType.Exp,
                    scale=0.125)
                meng = nc.vector if kt % 2 == 0 else nc.gpsimd
                for hh in range(2):
                    meng.tensor_mul(e_sb[:, hh, :], e_sb[:, hh, :],
                                    mask_sb[:, kt, :])
                return e_sb

            # single persistent Z bank; sweeps alternate 8-col groups
            z_bank = psZ.tile([128, QCS], F32, tag="z", name="z_bank")

            def start_sweep(qc, j):
                n = 4 * qc + j
                cps = psC.tile([128, QCS], F32, tag="c", name="cps", bufs=2)
                zoff = 256 + 8 * (n % 2)
                return {"qc": qc, "j": j, "c": cps, "z": z_bank, "zoff": zoff,
                        "n": 0}

            def emit_cz(sw, e_sb, kt):
                """C and Z matmuls for one kt of the lagged sweep."""
                j, cps = sw["j"], sw["c"]
                zb, zoff = sw["z"], sw["zoff"]
                for hh in range(2):
                    h = 2 * j + hh
                    for qsub in range(4):
                        idx = hh * 4 + qsub
                        est = e_sb[:, hh, qsub * 128:(qsub + 1) * 128]
                        first = (sw["n"] == 0)
                        sw["n"] += 1
                        nc.tensor.matmul(
                            cps[:, idx * 64:(idx + 1) * 64], est,
                            v_sb[kt][:, h * 65:h * 65 + 64],
                            start=first, stop=(kt == KT - 1 and idx == 7),
                            skip_group_check=True)
                        nc.tensor.matmul(
                            zb[:, zoff + idx:zoff + idx + 1], est,
                            v_sb[kt][:, h * 65 + 64:h * 65 + 65],
                            start=False, stop=(kt == KT - 1 and idx == 7),
                            skip_group_check=True)

            def emit_sweep_epilogue(sw):
                """recip Z, normalize -> bf16, PE transpose (into the C bank
                after its stripes are consumed), cpair copy."""
                qc, j, cps = sw["qc"], sw["j"], sw["c"]
                zb, zoff = sw["z"], sw["zoff"]
                rz = wp.tile([128, 8], F32, tag="rz", name="rz", bufs=2)
                nc.vector.reciprocal_approx_fast(
                    out=rz[:], in_=zb[:, zoff:zoff + 8])
                cn = wp.tile([128, 8, 64], BF16, tag="cn", name="cn", bufs=2)
                for idx in range(8):
                    nc.vector.tensor_scalar(
                        cn[:, idx, :], cps[:, idx * 64:(idx + 1) * 64],
                        rz[:, idx:idx + 1], 0.0,
                        mybir.AluOpType.mult, mybir.AluOpType.add)
                # transposes stage through a transient psO-ring tile: the
                # first one's start=True zeroes the bank, the rest rely on
                # the pending-zero it leaves (same pattern as Z/C banks)
                ct = psO.tile([128, QCS], BF16, tag="po", name="ct")
                first = True
                for hh in range(2):
                    for qsub in range(4):
                        idx = hh * 4 + qsub
                        nc.tensor.matmul(
                            ct[hh * 64:hh * 64 + 64,
                               qsub * 128:(qsub + 1) * 128],
                            cn[:, idx, :], id_sb[:],
                            is_transpose=True, start=first, stop=True,
                            skip_group_check=True)
                        first = False
                cpair = wp.tile([128, QCS], BF16, tag="cpair", name="cpair",
                                bufs=5)
                nc.vector.tensor_copy(cpair[:], ct[:])
                cpairs[(qc, j)] = cpair

            def emit_outproj_ot(qc, ot):
                cp = [cpairs[(qc, j)] for j in range(4)]
                po = psO.tile([128, QCS], F32, tag="po", name="po")
                for j in range(4):
                    nc.tensor.matmul(
                        po[:], wo_sb[:, j, ot * 128:(ot + 1) * 128],
                        cp[j][:], start=(j == 0), stop=(j == 3))
                o_sb = wp.tile([128, QCS], BF16, tag="o", name="o_sb", bufs=3)
                nc.vector.tensor_scalar(
                    o_sb[:], po[:], 1.0, vr_sb[:, ot:ot + 1],
                    mybir.AluOpType.mult, mybir.AluOpType.add)
                eng = nc.gpsimd if ot % 2 else nc.sync
                eng.dma_start(
                    outT[ot * 128:(ot + 1) * 128, qc * QCS:(qc + 1) * QCS],
                    o_sb[:])

            # ---- budgeted PE work queue ----
            # Items: (cost_ns, thunk). Emitted in FIFO order, paced so each
            # slot adds at most ~BUDGET ns of PE work on top of the scores.
            from collections import deque
            work = deque()
            BUDGET = 800.0
            CAP = 2000.0
            allowance = [0.0]

            def kt_(f, *a, **kw):
                return lambda: f(*a, **kw)

            def push(cost, thunk):
                work.append((cost, thunk))

            def drain_slot():
                allowance[0] = min(allowance[0] + BUDGET, CAP)
                while work and allowance[0] >= work[0][0]:
                    cost, thunk = work.popleft()
                    allowance[0] -= cost
                    thunk()

            def drain_all():
                while work:
                    _, thunk = work.popleft()
                    thunk()

            # fixed j0 slot tasks: the K tiles attention depends on (hard
            # deadlines), chunk-paced with their ring DMAs. Fixed slot k
            # fires right after scores(k+1) is emitted; the sweep-0 half of
            # chunk c must land before scores(kt=4c).
            fixed = {}
            fixed[(0, 0, 0)] = [kt_(emit_kproj_tile, 1, 0, 0)]
            fixed[(0, 0, 1)] = [kt_(emit_kproj_tile, 1, 0, 1)]
            fixed[(0, 0, 2)] = [kt_(emit_kproj_tile, 0, 1, 0)]
            fixed[(0, 0, 3)] = [kt_(emit_kproj_tile, 0, 1, 1)]
            fixed[(0, 0, 4)] = [kt_(xdma, nc.sync, xk_chunks, xk8, 3, "xk"),
                                kt_(emit_kproj_tile, 2, 0, 0)]
            fixed[(0, 0, 5)] = [kt_(emit_kproj_tile, 2, 0, 1)]
            fixed[(0, 0, 6)] = [kt_(emit_kproj_tile, 1, 1, 0)]
            fixed[(0, 0, 7)] = [kt_(emit_kproj_tile, 1, 1, 1)]
            fixed[(0, 0, 8)] = [kt_(emit_kproj_tile, 3, 0, 0)]
            fixed[(0, 0, 9)] = [kt_(emit_kproj_tile, 3, 0, 1)]
            fixed[(0, 0, 10)] = [kt_(emit_kproj_tile, 2, 1, 0)]
            fixed[(0, 0, 11)] = [kt_(emit_kproj_tile, 2, 1, 1)]
            fixed[(0, 0, 12)] = [kt_(emit_kproj_tile, 3, 1, 0)]
            fixed[(0, 0, 13)] = [kt_(emit_kproj_tile, 3, 1, 1)]
            fixed[(0, 0, 14)] = [kt_(emit_qproj_tile, 0, 1, 0)]
            fixed[(0, 0, 15)] = [kt_(emit_qproj_tile, 0, 1, 1)]

            for qcn in range(1, QCN):
                eng_x = nc.gpsimd if qcn % 2 else nc.sync
                fixed[(qcn - 1, 1, 12)] = [
                    kt_(xdma, eng_x, xq_chunks, xq8, qcn, "xq")]

            TILE_NS = 1300.0
            CZ_NS = 230.0
            EPI_NS = 900.0
            OT_NS = 880.0

            epi_count = [0]
            oa_tiles = {}

            def emit_outproj_partial(ot):
                """qc3 ots, j0-j2 partial accumulated early -> bf16 SBUF"""
                cp = [cpairs[(QCN - 1, j)] for j in range(3)]
                po = psO.tile([128, QCS], F32, tag="po", name="po")
                for j in range(3):
                    nc.tensor.matmul(
                        po[:], wo_sb[:, j, ot * 128:(ot + 1) * 128],
                        cp[j][:], start=(j == 0), stop=(j == 2))
                oa = wp.tile([128, QCS], BF16, tag="oa", name="oa", bufs=8)
                nc.vector.tensor_scalar(
                    oa[:], po[:], 1.0, 0.0,
                    mybir.AluOpType.mult, mybir.AluOpType.add)
                oa_tiles[ot] = oa

            def emit_epi(sw):
                emit_sweep_epilogue(sw)
                epi_count[0] += 1
                qc, j = sw["qc"], sw["j"]
                if j == 3 and qc < QCN - 1:
                    for ot in range(8):
                        push(OT_NS, kt_(emit_outproj_ot, qc, ot))
                if qc == QCN - 1 and j == 2:
                    for ot in range(8):
                        push(OT_NS, kt_(emit_outproj_partial, ot))

            def push_sweep(sw):
                """queue the CZ phase + epilogue of a finished sweep
                (early sweeps only; later sweeps inline their CZ)."""
                qc, j = sw["qc"], sw["j"]
                zb, zoff = sw["z"], sw["zoff"]
                # zero this sweep's Z col-group (queued, so it lands after
                # the PREVIOUS same-group sweep's Z reads)
                push(0, kt_(nc.vector.memset, zb[:, zoff:zoff + 8], 0.0))
                if qc == 0 and j == 0:
                    # V tiles interleave just ahead of the CZ kts that
                    # consume them (V projection happens here, in the lag
                    # window, not in phase A)
                    for kt in range(KT):
                        push(TILE_NS / 2, kt_(emit_vproj_part, kt, 0))
                        push(TILE_NS / 2, kt_(emit_vproj_part, kt, 1))
                        push(CZ_NS, kt_(emit_cz, sw, sw["e"][kt], kt))
                        if kt == 3:
                            push(0, kt_(xdma, nc.gpsimd, xv_chunks, xv8,
                                        2, "xv"))
                        if kt == 7:
                            push(0, kt_(xdma, nc.sync, xv_chunks, xv8,
                                        3, "xv"))
                else:
                    for kt in range(KT):
                        push(CZ_NS, kt_(emit_cz, sw, sw["e"][kt], kt))
                push(EPI_NS, kt_(emit_epi, sw))

            INLINE_FROM = 4   # sweeps with index >= this inline their CZ
            LAG = 3
            mask_next = mask0
            for qc in range(QCN):
                mask_sb = mask_next
                if qc > 0:
                    # this qc's Q tiles must be emitted before its scores
                    while qproj_done.get(qc, 0) < 4:
                        _, thunk = work.popleft()
                        thunk()
                for j in range(4):
                    n = 4 * qc + j
                    q8_t = q8_tiles[qc]
                    sw = start_sweep(qc, j)
                    # no accumulated burst right at a window boundary
                    allowance[0] = min(allowance[0], 400.0)
                    inline = (n >= INLINE_FROM)
                    if inline:
                        # predecessors of this sweep's C bank and Z group
                        # must be fully consumed before we touch them
                        while epi_count[0] < n - 1:
                            _, thunk = work.popleft()
                            thunk()
                        nc.vector.memset(
                            z_bank[:, sw["zoff"]:sw["zoff"] + 8], 0.0)
                    for kt in range(KT):
                        e_sb = emit_scores_exp_mask(qc, j, kt, mask_sb, q8_t)
                        sw.setdefault("e", []).append(e_sb)
                        # lookahead: scores/exp/mask of kt are emitted before
                        # the budgeted work of kt-1's slot
                        if kt > 0:
                            for thunk in fixed.pop((qc, j, kt - 1), ()):
                                thunk()
                            drain_slot()
                        if inline and kt >= LAG:
                            emit_cz(sw, sw["e"][kt - LAG], kt - LAG)
                        if j == 3 and kt == 3 and qc + 1 < QCN:
                            mask_next = load_mask(qc + 1)
                        if j == 2 and kt in (2, 5, 8, 11) and qc + 1 < QCN:
                            push(TILE_NS, kt_(emit_qproj_tile, qc + 1,
                                              (kt - 2) // 6, ((kt - 2) // 3) % 2))

                    for thunk in fixed.pop((qc, j, KT - 1), ()):
                        thunk()
                    drain_slot()
                    if inline:
                        for kt in range(KT - LAG, KT):
                            emit_cz(sw, sw["e"][kt], kt)
                        emit_epi(sw)
                    else:
                        push_sweep(sw)
                q8_tiles.pop(qc)
            drain_all()
            # final output projection (qc3): j3 delta + identity-matmul
            # fold of the staged j0-j2 partial, epilogue on the now-idle
            # Act engine, pipelined through both free PSUM rings
            for ot in range(8):
                if ot % 2:
                    po = psO.tile([128, QCS], F32, tag="po", name="po2")
                else:
                    po2 = psS.tile([128, 2, QCS], F32, tag="sps", name="po2",
                                   bufs=2)
                    po = po2[:, 0, :]
                nc.tensor.matmul(
                    po[:], wo_sb[:, 3, ot * 128:(ot + 1) * 128],
                    cpairs[(QCN - 1, 3)][:], start=True, stop=False)
                nc.tensor.matmul(
                    po[:], id_sb[:], oa_tiles[ot][:],
                    start=False, stop=True)
                o_sb = wp.tile([128, QCS], BF16, tag="o", name="o_sb", bufs=3)
                nc.scalar.activation(
                    o_sb[:], po[:], mybir.ActivationFunctionType.Identity,
                    bias=vr_sb[:, ot:ot + 1])
                eng = nc.gpsimd if ot % 2 else nc.sync
                eng.dma_start(
                    outT[ot * 128:(ot + 1) * 128,
                         (QCN - 1) * QCS:QCN * QCS],
                    o_sb[:])

    nc.finalize()
    return nc


_NC_CACHE = None


def _get_nc():
    global _NC_CACHE
    if _NC_CACHE is None:
        _NC_CACHE = build_nc()
    return _NC_CACHE


def _hi_lo_fp8(x):
    import ml_dtypes
    f8 = ml_dtypes.float8_e4m3
    hi = x.astype(f8)
    lo = (x - hi.astype(np.float32)).astype(f8)
    return hi, lo


def _x_prep(xT):
    """[D, S] f32 -> ([128, 4, 2, S] hi, lo) fp8 with din=i*256+s*128+p."""
    r = xT.reshape(4, 2, 128, xT.shape[1]).transpose(2, 0, 1, 3)
    return _hi_lo_fp8(np.ascontiguousarray(r))


def _w_prep(w_slice, perm):
    """w_slice [512 outf, 1024 din] -> ([128, 4, 2, 512] hi, lo) fp8 x16.
    Column c of the output = out-feature perm[c]."""
    w = (16.0 * w_slice[perm]).T          # [1024 din, 512 outcol]
    r = w.reshape(4, 2, 128, 512).transpose(2, 0, 1, 3)
    return _hi_lo_fp8(np.ascontiguousarray(r))


def shard_inputs(query, key, value, mask, wq, bq, wk, bk, wv, bv, wo, bo):
    import ml_dtypes
    bf = ml_dtypes.bfloat16

    query = np.asarray(query, np.float32)
    key = np.asarray(key, np.float32)
    value = np.asarray(value, np.float32)
    mask = np.asarray(mask)
    wq = np.asarray(wq, np.float32); bq = np.asarray(bq, np.float32)
    wk = np.asarray(wk, np.float32); bk = np.asarray(bk, np.float32)
    wv = np.asarray(wv, np.float32); bv = np.asarray(bv, np.float32)
    wo = np.asarray(wo, np.float32); bo = np.asarray(bo, np.float32)

    # out-feature permutation for Q/K: col (t, oc) -> f = (4s+oc//32)*64
    #  + 32*half + oc%32, t = 2s+half
    perm = np.empty(512, np.int64)
    for s_ in range(2):
        for half in range(2):
            t = 2 * s_ + half
            oc = np.arange(128)
            perm[t * 128:(t + 1) * 128] = ((4 * s_ + oc // 32) * 64
                                           + 32 * half + oc % 32)

    ident = np.eye(128, dtype=np.float32).astype(bf)
    maskT_b = [np.ascontiguousarray(mask[b].T).astype(bf) for b in range(B)]
    xp = {}
    for b in range(B):
        xp[b] = (
            _x_prep(np.ascontiguousarray(query[b].T)),
            _x_prep(np.ascontiguousarray(key[b].T)),
            _x_prep(np.ascontiguousarray(value[b].T)),
        )

    in_maps = []
    for c in range(NCORES):
        b, hg = divmod(c, HG)
        sl = slice(hg * DH, (hg + 1) * DH)
        wo_block = wo[:, sl]                       # [1024, 512]
        v_r = bv[sl] @ wo_block.T                  # [1024]
        if hg == 0:
            v_r = v_r + bo
        wq_hl = _w_prep(wq[sl], perm)
        wk_hl = _w_prep(wk[sl], perm)
        wv_hl = _w_prep(wv[sl], np.arange(512))
        bqk_arr = np.zeros((128, 8), np.float32)
        for t in range(4):
            p = np.arange(128)
            f = perm[t * 128 + p]
            bqk_arr[:, t] = bq[sl][f]
            bqk_arr[:, 4 + t] = bk[sl][f]
        (xq_hi, _), (xk_hi, _), (xv_hi, xv_lo) = xp[b]
        in_maps.append({
            "xq80": xq_hi,
            "xk80": xk_hi,
            "xv80": xv_hi, "xv81": xv_lo,
            "wq80": wq_hl[0], "wq81": wq_hl[1],
            "wk80": wk_hl[0], "wk81": wk_hl[1],
            "wv80": wv_hl[0], "wv81": wv_hl[1],
            "maskT": maskT_b[b],
            "woT": np.ascontiguousarray(wo_block.T).astype(bf),
            "bqk": bqk_arr,
            "vr2": np.ascontiguousarray(v_r.reshape(D // 128, 128).T),
            "ident": ident,
        })
    return in_maps


def combine_outputs(results):
    """results: list of per-core {"outT": [1024, 2048] bf16} -> [B, S, D]."""
    out = np.empty((B, S, D), np.float32)
    for b in range(B):
        acc = (results[2 * b]["outT"].astype(np.float32)
               + results[2 * b + 1]["outT"].astype(np.float32))
        out[b] = acc.T
    return out


def kernel(**inputs):
    from concourse.bass_utils import run_bass_kernel_spmd

    nc = _get_nc()
    in_maps = shard_inputs(**inputs)
    res = run_bass_kernel_spmd(nc, in_maps, list(range(NCORES)))
    return combine_outputs(res.results)
